# revision 13
# baseline (speedup 1.0000x reference)
"""ComplexLayerNorm Trainium2 kernel (8 NeuronCores, SPMD, C-sharded).

Math (see reference): per-feature 2x2 covariance whitening of (re, im) over
all B*C samples (centered with the batch-only mean mu_b), after subtracting
the complex mean over F, plus complex affine.

Sharding: C (=128) split 16-per-core, so the batch-sums T[c,f] = sum_b x and
mu_b are core-local and only 3 partial second-moment F-vectors (24 KB) need a
cross-core reduction (AllGather + local sum: cheaper than AllReduce).

v2 notes (cost-model-driven):
  * All PE matmuls stream their moving operand as bf16 (exact 0/1 selector)
    or float32r (same bits as fp32, 1 cyc/row when >=256 out cols) -- 4x the
    fp32 row rate.  Stationary operands are float32r bitcast views.
  * Phase A is f-quarter-pipelined: each 512-col f-quarter is loaded,
    transposed, and its covariance stats reduced while the next quarter
    loads, so the collective fires right after the last quarter.
  * AllGather (no 1.875x AllReduce multiplier) + 7 local DVE adds.
  * A3C/M3 DRAM bounce DMAs are spread across 4 queues.
  * Phase D emits x-matmuls before the K=3 correction matmul inside each
    PSUM accumulation group so PE can start before A3C is assembled.
"""

import numpy as np

import bass_rust
import concourse.bass as bass
import concourse.mybir as mybir
from concourse import tile
from concourse.bass_utils import run_bass_kernel_spmd


def split_multi_waits(nc):
    """The walrus build in this container allows only ONE sync-wait command
    per instruction; Tile emits several.  Split extras into preceding
    single-wait NoOps on the same engine (sequential waits == AND)."""
    cnt = 0
    for bb in nc.main_func.blocks:
        il = bb.instructions
        newlist = []
        changed = False
        for inst in list(il):
            si = inst.sync_info
            waits = list(si.on_wait) if si else []
            if len(waits) > 1:
                changed = True
                for w in waits[:-1]:
                    cnt += 1
                    nop = bass_rust.InstNoOp(name=f"I-wsplit-{cnt}")
                    nop.engine = inst.engine
                    nop.sync_info = mybir.SyncInfo(on_wait=[w], on_update=[])
                    newlist.append(nop)
                inst.sync_info = mybir.SyncInfo(
                    on_wait=[waits[-1]], on_update=list(si.on_update))
            newlist.append(inst)
        if changed:
            il[:] = newlist
    return cnt

FP = mybir.dt.float32
FPR = mybir.dt.float32r
BF = mybir.dt.bfloat16
AF = mybir.ActivationFunctionType
OP = mybir.AluOpType

B, C, F = 64, 128, 2048
NCORES = 8
CSH = C // NCORES           # 16 channels per core
BC = B * CSH                # 1024 sample rows per core
NFT = F // 128              # 16 f-chunks
NBC = BC // 128             # 8 bc-chunks
NQ = 4                      # f-quarters (4 f-chunks each)
EPS = 1e-4
NM1 = float(B * C - 1)      # 8191


def r(ap):
    """float32r view of an fp32 AP (same bits; 4x faster PE streaming)."""
    return ap.bitcast(FPR)


def build_bass():
    nc = bass.Bass()

    x_r = nc.dram_tensor("x_r", [BC, F], FPR, kind="ExternalInput")
    x_i = nc.dram_tensor("x_i", [BC, F], FPR, kind="ExternalInput")
    # gamma pre-tiled on host to (128, NFT): tile[p, t] = gamma[128*t + p]
    g_r = nc.dram_tensor("g_r", [128, NFT], FP, kind="ExternalInput")
    g_i = nc.dram_tensor("g_i", [128, NFT], FP, kind="ExternalInput")
    # beta interleaved on host: (1, 4096) = [b_r[0], b_i[0], b_r[1], ...]
    beta_ilv = nc.dram_tensor("beta_ilv", [1, 2 * F], FPR, kind="ExternalInput")
    ident = nc.dram_tensor("ident", [128, 128], FP, kind="ExternalInput")
    identsel = nc.dram_tensor("identsel", [128, 256], FPR,
                              kind="ExternalInput")
    onesF = nc.dram_tensor("onesF", [128, 1], FPR, kind="ExternalInput")

    out = nc.dram_tensor("out", [BC, 2 * F], FP, kind="ExternalOutput")
    dbg = nc.dram_tensor("dbg", [128, 3 * NFT], FP, kind="ExternalOutput")

    with tile.TileContext(nc) as tc:
        with (
            tc.tile_pool(name="big", bufs=1) as big,
            tc.tile_pool(name="small", bufs=1) as small,
            tc.tile_pool(name="wpool", bufs=2) as wpool,
            tc.tile_pool(name="stage", bufs=2) as stage,
            tc.tile_pool(name="dram", bufs=1, space="DRAM") as dram,
        ):
            # ---- constants to SBUF
            ident_t = small.tile([128, 128], FP, tag="ident")
            nc.sync.dma_start(ident_t[:], ident[:])
            # [ident | sel | zero-pad] padded to 256 cols: streamed as fp32r
            # the >=256-col moving operand runs at 1 cyc/row (4x fp32), with
            # bit-exact fp32 numerics.
            identsel_t = small.tile([128, 256], FPR, tag="identsel")
            nc.sync.dma_start(identsel_t[:], identsel[:])
            onesF_t = small.tile([128, 1], FPR, tag="onesF")
            nc.sync.dma_start(onesF_t[:], onesF[:])
            g_r_t = small.tile([128, NFT], FP, tag="g_r")
            nc.sync.dma_start(g_r_t[:], g_r[:])
            g_i_t = small.tile([128, NFT], FP, tag="g_i")
            nc.sync.dma_start(g_i_t[:], g_i[:])

            # ---- persistent: x transposed, xT[p, 1024*t + j] = x[j, 128*t+p]
            xT_r = big.tile([128, NFT * BC], FPR, tag="xT_r")
            xT_i = big.tile([128, NFT * BC], FPR, tag="xT_i")

            from contextlib import ExitStack
            _stk = ExitStack()
            xin = _stk.enter_context(tc.tile_pool(name="xin", bufs=3))
            scratch = _stk.enter_context(tc.tile_pool(name="scratch", bufs=2))
            ps_xt = _stk.enter_context(
                tc.tile_pool(name="ps_xt", bufs=2, space="PSUM"))

            # T[c,f] accumulators in SBUF: [p (=f in chunk), 16*t + c].
            T_r_sb = small.tile([128, NFT * CSH], FP, tag="T_r_sb")
            T_i_sb = small.tile([128, NFT * CSH], FP, tag="T_i_sb")

            # per-f stats, packed [S_rr | S_ri | S_ii] column-chunks
            S_rr = small.tile([128, NFT], FP, tag="S_rr")
            S_ri = small.tile([128, NFT], FP, tag="S_ri")
            S_ii = small.tile([128, NFT], FP, tag="S_ii")
            corr_rr = small.tile([128, NFT], FP, tag="corr_rr")
            corr_ri = small.tile([128, NFT], FP, tag="corr_ri")
            corr_ii = small.tile([128, NFT], FP, tag="corr_ii")
            partial = small.tile([128, 3 * NFT], FP, tag="partial")

            # ---- Phase A: f-quarter-pipelined load + PE transpose + stats.
            # Quarter tg covers f-chunks 4*tg..4*tg+3 (512 f-columns).
            copy_flip = 0
            for tg in range(NQ):
                fsl = slice(512 * tg, 512 * (tg + 1))
                for b in range(NBC):
                    rsl = slice(128 * b, 128 * (b + 1))
                    xn_r = xin.tile([128, 512], FPR, tag="xn")
                    nc.sync.dma_start(xn_r[:], x_r[rsl, fsl])
                    xn_i = xin.tile([128, 512], FPR, tag="xn")
                    nc.scalar.dma_start(xn_i[:], x_i[rsl, fsl])
                    for xn, xT, T_sb in (
                        (xn_r, xT_r, T_r_sb), (xn_i, xT_i, T_i_sb)
                    ):
                        # one matmul per f-chunk: rhs [ident | sel] gives the
                        # 128-col transpose AND the 16-col T partial. regions
                        # at 256-col spacing so no mm output crosses a bank.
                        pxt = ps_xt.tile([128, 1024], FP, tag="pxt")
                        for tt in range(4):
                            t = 4 * tg + tt
                            nc.tensor.matmul(
                                pxt[:, 256 * tt:256 * (tt + 1)],
                                xn[:, 128 * tt:128 * (tt + 1)],
                                identsel_t[:],
                                start=True, stop=True,
                            )
                        pv = pxt[:].rearrange("p (a q) -> p a q", q=256)
                        dst = xT[:].rearrange("p (a q) -> p a q", q=1024)[
                            :, 4 * tg:4 * (tg + 1), 128 * b:128 * (b + 1)
                        ]
                        if copy_flip % 2 == 0:
                            nc.scalar.copy(dst, pv[:, :, 0:128])
                        else:
                            nc.vector.tensor_copy(dst, pv[:, :, 0:128])
                        copy_flip += 1
                        tdst = T_sb[:, 64 * tg:64 * (tg + 1)].rearrange(
                            "p (a q) -> p a q", q=CSH)
                        tsrc = pv[:, :, 128:128 + CSH]
                        if b == 0:
                            nc.vector.tensor_copy(tdst, tsrc)
                        else:
                            nc.vector.scalar_tensor_tensor(
                                out=tdst, in0=tsrc, scalar=1.0, in1=tdst,
                                op0=OP.mult, op1=OP.add,
                            )

                # quarter's xT rows are complete: second moments for its 4 t's
                for tt in range(4):
                    t = 4 * tg + tt
                    sl = slice(BC * t, BC * (t + 1))
                    sc1 = scratch.tile([128, BC], FP, tag="sq")
                    nc.scalar.activation(sc1[:], xT_r[:, sl].bitcast(FP),
                                         AF.Square,
                                         accum_out=S_rr[:, t:t + 1])
                    sc2 = scratch.tile([128, BC], FP, tag="sq")
                    nc.scalar.activation(sc2[:], xT_i[:, sl].bitcast(FP),
                                         AF.Square,
                                         accum_out=S_ii[:, t:t + 1])
                    sc3 = scratch.tile([128, BC], FP, tag="sq")
                    nc.vector.scalar_tensor_tensor(
                        out=sc3[:], in0=xT_r[:, sl].bitcast(FP), scalar=1.0,
                        in1=xT_i[:, sl].bitcast(FP), op0=OP.mult, op1=OP.mult,
                        accum_out=S_ri[:, t:t + 1],
                    )
                    # T quadratic correction for this t
                    tsl = slice(CSH * t, CSH * (t + 1))
                    ts1 = scratch.tile([128, CSH], FP, tag="tsq")
                    nc.scalar.activation(ts1[:], T_r_sb[:, tsl], AF.Square,
                                         accum_out=corr_rr[:, t:t + 1])
                    ts2 = scratch.tile([128, CSH], FP, tag="tsq")
                    nc.scalar.activation(ts2[:], T_i_sb[:, tsl], AF.Square,
                                         accum_out=corr_ii[:, t:t + 1])
                    ts3 = scratch.tile([128, CSH], FP, tag="tsq")
                    nc.vector.scalar_tensor_tensor(
                        out=ts3[:], in0=T_r_sb[:, tsl], scalar=1.0,
                        in1=T_i_sb[:, tsl], op0=OP.mult, op1=OP.mult,
                        accum_out=corr_ri[:, t:t + 1],
                    )
                # local partial covariance for this quarter's columns
                qsl = slice(4 * tg, 4 * (tg + 1))
                for j, (S, corr) in enumerate(
                    ((S_rr, corr_rr), (S_ri, corr_ri), (S_ii, corr_ii))
                ):
                    dstq = partial[:, NFT * j + 4 * tg:NFT * j + 4 * (tg + 1)]
                    nc.vector.scalar_tensor_tensor(
                        out=dstq, in0=corr[:, qsl], scalar=-1.0 / B,
                        in1=S[:, qsl], op0=OP.mult, op1=OP.add,
                    )
                    nc.vector.tensor_scalar(
                        out=dstq, in0=dstq, scalar1=1.0 / NM1, scalar2=None,
                        op0=OP.mult,
                    )

            # ---- AllGather partial covariances (24 KB in, 192 KB out), then
            # local sum of the 8 per-core partials.  AllGather avoids the
            # AllReduce's 1.875x fixed-cost multiplier.
            ar_in = dram.tile([128, 3 * NFT], FP, tag="ar_in")
            ar_out = dram.tile([NCORES * 128, 3 * NFT], FP, tag="ar_out")
            nc.sync.dma_start(ar_in[:], partial[:])
            nc.gpsimd.collective_compute(
                "AllGather", OP.bypass,
                replica_groups=[list(range(NCORES))],
                ins=[ar_in.opt()],
                outs=[ar_out.opt()],
            )
            covp = []
            for k in range(NCORES):
                cvk = scratch.tile([128, 3 * NFT], FP, tag=f"cv{k}",
                                   name=f"cv{k}")
                eng = (nc.sync, nc.scalar, nc.gpsimd)[k % 3]
                eng.dma_start(cvk[:], ar_out[128 * k:128 * (k + 1), :])
                covp.append(cvk)
            cov = small.tile([128, 3 * NFT], FP, tag="cov")
            nc.vector.tensor_tensor(out=cov[:], in0=covp[0][:], in1=covp[1][:],
                                    op=OP.add)
            for k in range(2, NCORES):
                nc.vector.tensor_tensor(out=cov[:], in0=cov[:], in1=covp[k][:],
                                        op=OP.add)

            nc.sync.dma_start(dbg[:], cov[:])

            # ---- Phase B: complex mean over F via PE ones-matmul on xT
            # (overlaps the collective -- no dependency on cov).
            _stk2 = ExitStack()
            ps_mean = _stk2.enter_context(
                tc.tile_pool(name="ps_mean", bufs=1, space="PSUM"))
            psm_r = ps_mean.tile([1, BC], FP, tag="psm_r")
            psm_i = ps_mean.tile([1, BC], FP, tag="psm_i")
            for xT, psm in ((xT_r, psm_r), (xT_i, psm_i)):
                for t in range(NFT):
                    for h in range(2):
                        nc.tensor.matmul(
                            psm[:, 512 * h:512 * (h + 1)],
                            onesF_t[:],
                            xT[:, BC * t + 512 * h:BC * t + 512 * (h + 1)],
                            start=(t == 0), stop=(t == NFT - 1),
                        )
            # M3 = [-mean_r; -mean_i; ones]  (3, 1024).  Engine ops cannot
            # write at partition offsets 1/2, so build rows at partition 0
            # and DMA them into place (3 queues in parallel).
            M3 = small.tile([3, BC], FPR, tag="M3")
            row0 = small.tile([1, BC], FPR, tag="rowtmp", name="row0")
            nc.vector.tensor_scalar(out=row0[:], in0=psm_r[:],
                                    scalar1=-1.0, scalar2=None, op0=OP.mult)
            nc.sync.dma_start(M3[0:1, :], row0[:])
            row1 = small.tile([1, BC], FPR, tag="rowtmp", name="row1")
            nc.vector.tensor_scalar(out=row1[:], in0=psm_i[:],
                                    scalar1=-1.0, scalar2=None, op0=OP.mult)
            nc.scalar.dma_start(M3[1:2, :], row1[:])
            row2 = small.tile([1, BC], FPR, tag="rowtmp", name="row2")
            nc.vector.tensor_scalar(out=row2[:], in0=psm_i[:],
                                    scalar1=0.0, scalar2=1.0, op0=OP.mult,
                                    op1=OP.add)
            nc.gpsimd.dma_start(M3[2:3, :], row2[:])
            _stk2.close()

            # ---- Phase C: closed-form 2x2 inverse sqrt, fold gamma -> A
            def stile(tag):
                return small.tile([128, NFT], FP, tag=tag, name=tag)

            arr, bri, cii = stile("arr"), stile("bri"), stile("cii")
            nc.vector.tensor_scalar(out=arr[:], in0=cov[:, 0:NFT],
                                    scalar1=EPS, scalar2=None, op0=OP.add)
            nc.vector.tensor_copy(bri[:], cov[:, NFT:2 * NFT])
            nc.vector.tensor_scalar(out=cii[:], in0=cov[:, 2 * NFT:3 * NFT],
                                    scalar1=EPS, scalar2=None, op0=OP.add)

            det, tmp = stile("det"), stile("tmp")
            nc.vector.tensor_tensor(out=det[:], in0=arr[:], in1=cii[:],
                                    op=OP.mult)
            nc.vector.tensor_tensor(out=tmp[:], in0=bri[:], in1=bri[:],
                                    op=OP.mult)
            nc.vector.tensor_tensor(out=det[:], in0=det[:], in1=tmp[:],
                                    op=OP.subtract)
            s_t = stile("s_t")
            nc.scalar.activation(s_t[:], det[:], AF.Sqrt)
            # tval = sqrt(a + c + 2 s)
            tsum = stile("tsum")
            nc.vector.tensor_tensor(out=tsum[:], in0=arr[:], in1=cii[:],
                                    op=OP.add)
            nc.vector.scalar_tensor_tensor(out=tsum[:], in0=s_t[:], scalar=2.0,
                                           in1=tsum[:], op0=OP.mult, op1=OP.add)
            tval = stile("tval")
            nc.scalar.activation(tval[:], tsum[:], AF.Sqrt)
            den, rden = stile("den"), stile("rden")
            nc.vector.tensor_tensor(out=den[:], in0=s_t[:], in1=tval[:],
                                    op=OP.mult)
            nc.vector.reciprocal(rden[:], den[:])

            w_rr, w_ii, wri_n = stile("w_rr"), stile("w_ii"), stile("wri_n")
            # w_rr = (c+s)*rden ; w_ii = (a+s)*rden ; w_ri = -b*rden = wri_n
            nc.vector.tensor_tensor(out=w_rr[:], in0=cii[:], in1=s_t[:],
                                    op=OP.add)
            nc.vector.tensor_tensor(out=w_rr[:], in0=w_rr[:], in1=rden[:],
                                    op=OP.mult)
            nc.vector.tensor_tensor(out=w_ii[:], in0=arr[:], in1=s_t[:],
                                    op=OP.add)
            nc.vector.tensor_tensor(out=w_ii[:], in0=w_ii[:], in1=rden[:],
                                    op=OP.mult)
            nc.vector.tensor_tensor(out=wri_n[:], in0=bri[:], in1=rden[:],
                                    op=OP.mult)
            nc.vector.tensor_scalar(out=wri_n[:], in0=wri_n[:], scalar1=-1.0,
                                    scalar2=None, op0=OP.mult)

            # A = G @ W,  G = [[g_r, -g_i], [g_i, g_r]], W = [[w_rr, w_ri],
            # [w_ri, w_ii]] with w_ri = wri_n
            def rtile(tag):
                return small.tile([128, NFT], FPR, tag=tag, name=tag)

            a_rr, a_ri = rtile("a_rr"), rtile("a_ri")
            a_ir, a_ii = rtile("a_ir"), rtile("a_ii")
            u, v = stile("u"), stile("v")
            # a_rr = g_r*w_rr - g_i*w_ri
            nc.vector.tensor_tensor(out=u[:], in0=g_r_t[:], in1=w_rr[:],
                                    op=OP.mult)
            nc.vector.tensor_tensor(out=v[:], in0=g_i_t[:], in1=wri_n[:],
                                    op=OP.mult)
            nc.vector.tensor_tensor(out=a_rr[:], in0=u[:], in1=v[:],
                                    op=OP.subtract)
            # a_ri = g_r*w_ri - g_i*w_ii
            nc.vector.tensor_tensor(out=u[:], in0=g_r_t[:], in1=wri_n[:],
                                    op=OP.mult)
            nc.vector.tensor_tensor(out=v[:], in0=g_i_t[:], in1=w_ii[:],
                                    op=OP.mult)
            nc.vector.tensor_tensor(out=a_ri[:], in0=u[:], in1=v[:],
                                    op=OP.subtract)
            # a_ir = g_i*w_rr + g_r*w_ri
            nc.vector.tensor_tensor(out=u[:], in0=g_i_t[:], in1=w_rr[:],
                                    op=OP.mult)
            nc.vector.tensor_tensor(out=v[:], in0=g_r_t[:], in1=wri_n[:],
                                    op=OP.mult)
            nc.vector.tensor_tensor(out=a_ir[:], in0=u[:], in1=v[:],
                                    op=OP.add)
            # a_ii = g_i*w_ri + g_r*w_ii
            nc.vector.tensor_tensor(out=u[:], in0=g_i_t[:], in1=wri_n[:],
                                    op=OP.mult)
            nc.vector.tensor_tensor(out=v[:], in0=g_r_t[:], in1=w_ii[:],
                                    op=OP.mult)
            nc.vector.tensor_tensor(out=a_ii[:], in0=u[:], in1=v[:],
                                    op=OP.add)

            # ---- A3C rhs for the K=3 correction matmul: (3, 4096)
            # row0[2f+c] = (a_rr, a_ir)[c][f]; row1: (a_ri, a_ii); row2: beta
            A3C = small.tile([3, 2 * F], FPR, tag="A3C")
            rings = (nc.sync, nc.scalar, nc.gpsimd, nc.scalar)
            for row, (ev, od) in enumerate(((a_rr, a_ir), (a_ri, a_ii))):
                for cpar, srctile in ((0, ev), (1, od)):
                    # bounce through DRAM; read back in f-major order with a
                    # strided AP.  dram layout: addr(p, t) = 16*p + t.
                    eng = rings[2 * row + cpar]
                    dbuf = dram.tile([128, NFT], FPR, tag=f"dbuf{row}{cpar}",
                                     name=f"dbuf{row}{cpar}")
                    eng.dma_start(dbuf[:], srctile[:])
                    # src iterates (t, p): steps [[1, 16], [16, 128]]
                    src = dbuf[:].rearrange("p t -> (p t)").rearrange(
                        "(p t) -> t p", p=128, t=NFT
                    )
                    dst = A3C[row:row + 1, cpar::2].rearrange(
                        "z (t p) -> z t p", t=NFT, p=128
                    )
                    eng.dma_start(dst, src)
            nc.sync.dma_start(A3C[2:3, :], beta_ilv[:])

            # release phase-A pools (xin/scratch SBUF, transpose/T PSUM)
            _stk.close()

            # ---- Phase D: apply.  t-outer; W built on the fly.  Inside each
            # PSUM group the x-matmuls come first (start=True per region) and
            # the A3C correction last, so PE needn't wait for A3C.
            _stk3 = ExitStack()
            ps_o = _stk3.enter_context(
                tc.tile_pool(name="ps_o", bufs=8, space="PSUM"))
            for t2 in range(NFT // 2):
                ta, tb = 2 * t2, 2 * t2 + 1
                Ws = []
                for t in (ta, tb):
                    W_r = wpool.tile([128, 256], FPR, tag="W_r",
                                     name=f"W_r_{t}")
                    W_i = wpool.tile([128, 256], FPR, tag="W_i",
                                     name=f"W_i_{t}")
                    for W, (ev, od) in ((W_r, (a_rr, a_ir)),
                                        (W_i, (a_ri, a_ii))):
                        Wv = W[:].rearrange("p (g c) -> p g c", c=2)
                        nc.vector.tensor_scalar(
                            out=Wv[:, :, 0], in0=ident_t[:],
                            scalar1=ev[:, t:t + 1].bitcast(FP), scalar2=None,
                            op0=OP.mult,
                        )
                        nc.scalar.activation(
                            Wv[:, :, 1], ident_t[:], AF.Copy,
                            scale=od[:, t:t + 1].bitcast(FP),
                        )
                    Ws.append((W_r, W_i))
                for bh in range(2):
                    stg = stage.tile([128, 4 * 512], FP, tag="stg")
                    pos = []
                    for bb in range(4):
                        b = 4 * bh + bb
                        po = ps_o.tile([128, 512], FP, tag="po")
                        pos.append(po)
                        nc.tensor.matmul(
                            po[:],
                            M3[:, 128 * b:128 * (b + 1)],
                            A3C[:, 512 * t2:512 * (t2 + 1)],
                            start=True, stop=False,
                        )
                        for j, t in enumerate((ta, tb)):
                            W_r, W_i = Ws[j]
                            sl = slice(BC * t + 128 * b,
                                       BC * t + 128 * (b + 1))
                            nc.tensor.matmul(
                                po[:, 256 * j:256 * (j + 1)],
                                xT_r[:, sl], W_r[:],
                                start=False, stop=False,
                            )
                            nc.tensor.matmul(
                                po[:, 256 * j:256 * (j + 1)],
                                xT_i[:, sl], W_i[:],
                                start=False, stop=(j == 1),
                            )
                    for bb in range(4):
                        po = pos[bb]
                        if (4 * bh + bb) % 2 == 0:
                            nc.vector.tensor_copy(
                                stg[:, 512 * bb:512 * (bb + 1)], po[:])
                        else:
                            nc.scalar.copy(
                                stg[:, 512 * bb:512 * (bb + 1)], po[:])
                    # 1 MB store: rows (b, p) -> out[128*b + p, 512*t2:+512]
                    dst = out.rearrange("(a p) f -> p a f", p=128)[
                        :, 4 * bh:4 * (bh + 1), 512 * t2:512 * (t2 + 1)
                    ]
                    src = stg[:].rearrange("p (a q) -> p a q", q=512)
                    seng = (nc.sync, nc.scalar, nc.gpsimd, nc.sync,
                            nc.scalar, nc.gpsimd, nc.sync, nc.gpsimd)[
                        (2 * t2 + bh) % 8]
                    seng.dma_start(dst, src)
            _stk3.close()

    split_multi_waits(nc)
    return nc


_CACHE = {}


def _get_nc():
    if "nc" not in _CACHE:
        _CACHE["nc"] = build_bass()
    return _CACHE["nc"]


def _constants():
    if "consts" not in _CACHE:
        sel = np.zeros((128, CSH), dtype=np.float32)
        for p in range(128):
            sel[p, p % CSH] = 1.0
        _CACHE["consts"] = {
            "ident": np.eye(128, dtype=np.float32),
            "identsel": np.ascontiguousarray(np.concatenate(
                [np.eye(128, dtype=np.float32), sel,
                 np.zeros((128, 112), dtype=np.float32)], axis=1)),
            "onesF": np.full((128, 1), 1.0 / F, dtype=np.float32),
        }
    return _CACHE["consts"]


def kernel(x_real, x_imag, gamma_r, gamma_i, beta_r, beta_i):
    x_real = np.ascontiguousarray(x_real, dtype=np.float32)
    x_imag = np.ascontiguousarray(x_imag, dtype=np.float32)
    gamma_r = np.asarray(gamma_r, dtype=np.float32)
    gamma_i = np.asarray(gamma_i, dtype=np.float32)
    beta_r = np.asarray(beta_r, dtype=np.float32)
    beta_i = np.asarray(beta_i, dtype=np.float32)

    nc = _get_nc()
    consts = _constants()
    g_r_t = np.ascontiguousarray(gamma_r.reshape(NFT, 128).T)
    g_i_t = np.ascontiguousarray(gamma_i.reshape(NFT, 128).T)
    beta_ilv = np.ascontiguousarray(
        np.stack([beta_r, beta_i], axis=-1).reshape(1, 2 * F)
    )

    in_maps = []
    for k in range(NCORES):
        cs = slice(CSH * k, CSH * (k + 1))
        in_maps.append({
            "x_r": np.ascontiguousarray(
                x_real[:, cs, :].reshape(BC, F)),
            "x_i": np.ascontiguousarray(
                x_imag[:, cs, :].reshape(BC, F)),
            "g_r": g_r_t, "g_i": g_i_t, "beta_ilv": beta_ilv,
            **consts,
        })

    res = run_bass_kernel_spmd(nc, in_maps, list(range(NCORES)))

    full = np.empty((B, C, F, 2), dtype=np.float32)
    for k in range(NCORES):
        full[:, CSH * k:CSH * (k + 1)] = (
            res.results[k]["out"].reshape(B, CSH, F, 2)
        )
    return full


# revision 14
# speedup vs baseline: 1.0070x; 1.0070x over previous
"""ComplexLayerNorm Trainium2 kernel (8 NeuronCores, SPMD, C-sharded).

Math (see reference): per-feature 2x2 covariance whitening of (re, im) over
all B*C samples (centered with the batch-only mean mu_b), after subtracting
the complex mean over F, plus complex affine.

Sharding: C (=128) split 16-per-core, so the batch-sums T[c,f] = sum_b x and
mu_b are core-local and only 3 partial second-moment F-vectors (24 KB) need a
cross-core reduction (AllGather + local sum: cheaper than AllReduce).

v2 notes (cost-model-driven):
  * All PE matmuls stream their moving operand as bf16 (exact 0/1 selector)
    or float32r (same bits as fp32, 1 cyc/row when >=256 out cols) -- 4x the
    fp32 row rate.  Stationary operands are float32r bitcast views.
  * Phase A is f-quarter-pipelined: each 512-col f-quarter is loaded,
    transposed, and its covariance stats reduced while the next quarter
    loads, so the collective fires right after the last quarter.
  * AllGather (no 1.875x AllReduce multiplier) + 7 local DVE adds.
  * A3C/M3 DRAM bounce DMAs are spread across 4 queues.
  * Phase D emits x-matmuls before the K=3 correction matmul inside each
    PSUM accumulation group so PE can start before A3C is assembled.
"""

import numpy as np

import bass_rust
import concourse.bass as bass
import concourse.mybir as mybir
from concourse import tile
from concourse.bass_utils import run_bass_kernel_spmd


def split_multi_waits(nc):
    """The walrus build in this container allows only ONE sync-wait command
    per instruction; Tile emits several.  Split extras into preceding
    single-wait NoOps on the same engine (sequential waits == AND)."""
    cnt = 0
    for bb in nc.main_func.blocks:
        il = bb.instructions
        newlist = []
        changed = False
        for inst in list(il):
            si = inst.sync_info
            waits = list(si.on_wait) if si else []
            if len(waits) > 1:
                changed = True
                for w in waits[:-1]:
                    cnt += 1
                    nop = bass_rust.InstNoOp(name=f"I-wsplit-{cnt}")
                    nop.engine = inst.engine
                    nop.sync_info = mybir.SyncInfo(on_wait=[w], on_update=[])
                    newlist.append(nop)
                inst.sync_info = mybir.SyncInfo(
                    on_wait=[waits[-1]], on_update=list(si.on_update))
            newlist.append(inst)
        if changed:
            il[:] = newlist
    return cnt

FP = mybir.dt.float32
FPR = mybir.dt.float32r
BF = mybir.dt.bfloat16
AF = mybir.ActivationFunctionType
OP = mybir.AluOpType

B, C, F = 64, 128, 2048
NCORES = 8
CSH = C // NCORES           # 16 channels per core
BC = B * CSH                # 1024 sample rows per core
NFT = F // 128              # 16 f-chunks
NBC = BC // 128             # 8 bc-chunks
NQ = 4                      # f-quarters (4 f-chunks each)
EPS = 1e-4
NM1 = float(B * C - 1)      # 8191


def r(ap):
    """float32r view of an fp32 AP (same bits; 4x faster PE streaming)."""
    return ap.bitcast(FPR)


def build_bass():
    nc = bass.Bass()

    x_r = nc.dram_tensor("x_r", [BC, F], FPR, kind="ExternalInput")
    x_i = nc.dram_tensor("x_i", [BC, F], FPR, kind="ExternalInput")
    # gamma pre-tiled on host to (128, NFT): tile[p, t] = gamma[128*t + p]
    g_r = nc.dram_tensor("g_r", [128, NFT], FP, kind="ExternalInput")
    g_i = nc.dram_tensor("g_i", [128, NFT], FP, kind="ExternalInput")
    # beta interleaved on host: (1, 4096) = [b_r[0], b_i[0], b_r[1], ...]
    beta_ilv = nc.dram_tensor("beta_ilv", [1, 2 * F], FPR, kind="ExternalInput")
    ident = nc.dram_tensor("ident", [128, 128], FP, kind="ExternalInput")
    identsel = nc.dram_tensor("identsel", [128, 256], FPR,
                              kind="ExternalInput")
    onesF = nc.dram_tensor("onesF", [128, 1], FPR, kind="ExternalInput")

    out = nc.dram_tensor("out", [BC, 2 * F], FP, kind="ExternalOutput")
    dbg = nc.dram_tensor("dbg", [128, 3 * NFT], FP, kind="ExternalOutput")

    with tile.TileContext(nc) as tc:
        with (
            tc.tile_pool(name="big", bufs=1) as big,
            tc.tile_pool(name="small", bufs=1) as small,
            tc.tile_pool(name="wpool", bufs=2) as wpool,
            tc.tile_pool(name="stage", bufs=2) as stage,
            tc.tile_pool(name="dram", bufs=1, space="DRAM") as dram,
        ):
            # ---- constants to SBUF
            ident_t = small.tile([128, 128], FP, tag="ident")
            nc.sync.dma_start(ident_t[:], ident[:])
            # [ident | sel | zero-pad] padded to 256 cols: streamed as fp32r
            # the >=256-col moving operand runs at 1 cyc/row (4x fp32), with
            # bit-exact fp32 numerics.
            identsel_t = small.tile([128, 256], FPR, tag="identsel")
            nc.sync.dma_start(identsel_t[:], identsel[:])
            onesF_t = small.tile([128, 1], FPR, tag="onesF")
            nc.sync.dma_start(onesF_t[:], onesF[:])
            g_r_t = small.tile([128, NFT], FP, tag="g_r")
            nc.sync.dma_start(g_r_t[:], g_r[:])
            g_i_t = small.tile([128, NFT], FP, tag="g_i")
            nc.sync.dma_start(g_i_t[:], g_i[:])

            # ---- persistent: x transposed, xT[p, 1024*t + j] = x[j, 128*t+p]
            xT_r = big.tile([128, NFT * BC], FPR, tag="xT_r")
            xT_i = big.tile([128, NFT * BC], FPR, tag="xT_i")

            from contextlib import ExitStack
            _stk = ExitStack()
            xin = _stk.enter_context(tc.tile_pool(name="xin", bufs=3))
            scratch = _stk.enter_context(tc.tile_pool(name="scratch", bufs=2))
            ps_xt = _stk.enter_context(
                tc.tile_pool(name="ps_xt", bufs=2, space="PSUM"))

            # T[c,f] accumulators in SBUF: [p (=f in chunk), 16*t + c].
            T_r_sb = small.tile([128, NFT * CSH], FP, tag="T_r_sb")
            T_i_sb = small.tile([128, NFT * CSH], FP, tag="T_i_sb")

            # per-f stats, packed [S_rr | S_ri | S_ii] column-chunks
            S_rr = small.tile([128, NFT], FP, tag="S_rr")
            S_ri = small.tile([128, NFT], FP, tag="S_ri")
            S_ii = small.tile([128, NFT], FP, tag="S_ii")
            corr_rr = small.tile([128, NFT], FP, tag="corr_rr")
            corr_ri = small.tile([128, NFT], FP, tag="corr_ri")
            corr_ii = small.tile([128, NFT], FP, tag="corr_ii")
            partial = small.tile([128, 3 * NFT], FP, tag="partial")

            # ---- Phase A: f-quarter-pipelined load + PE transpose + stats.
            # Quarter tg covers f-chunks 4*tg..4*tg+3 (512 f-columns).
            copy_flip = 0
            for tg in range(NQ):
                fsl = slice(512 * tg, 512 * (tg + 1))
                for b in range(NBC):
                    rsl = slice(128 * b, 128 * (b + 1))
                    xn_r = xin.tile([128, 512], FPR, tag="xn")
                    nc.sync.dma_start(xn_r[:], x_r[rsl, fsl])
                    xn_i = xin.tile([128, 512], FPR, tag="xn")
                    nc.scalar.dma_start(xn_i[:], x_i[rsl, fsl])
                    for xn, xT, T_sb in (
                        (xn_r, xT_r, T_r_sb), (xn_i, xT_i, T_i_sb)
                    ):
                        # one matmul per f-chunk: rhs [ident | sel] gives the
                        # 128-col transpose AND the 16-col T partial. regions
                        # at 256-col spacing so no mm output crosses a bank.
                        pxt = ps_xt.tile([128, 1024], FP, tag="pxt")
                        for tt in range(4):
                            t = 4 * tg + tt
                            nc.tensor.matmul(
                                pxt[:, 256 * tt:256 * (tt + 1)],
                                xn[:, 128 * tt:128 * (tt + 1)],
                                identsel_t[:],
                                start=True, stop=True,
                            )
                        pv = pxt[:].rearrange("p (a q) -> p a q", q=256)
                        dst = xT[:].rearrange("p (a q) -> p a q", q=1024)[
                            :, 4 * tg:4 * (tg + 1), 128 * b:128 * (b + 1)
                        ]
                        nc.scalar.copy(dst, pv[:, :, 0:128])
                        copy_flip += 1
                        tdst = T_sb[:, 64 * tg:64 * (tg + 1)].rearrange(
                            "p (a q) -> p a q", q=CSH)
                        tsrc = pv[:, :, 128:128 + CSH]
                        if b == 0:
                            nc.vector.tensor_copy(tdst, tsrc)
                        else:
                            nc.vector.scalar_tensor_tensor(
                                out=tdst, in0=tsrc, scalar=1.0, in1=tdst,
                                op0=OP.mult, op1=OP.add,
                            )

                # quarter's xT rows are complete: second moments for its 4 t's
                for tt in range(4):
                    t = 4 * tg + tt
                    sl = slice(BC * t, BC * (t + 1))
                    sc1 = scratch.tile([128, BC], FP, tag="sq")
                    nc.vector.scalar_tensor_tensor(
                        out=sc1[:], in0=xT_r[:, sl].bitcast(FP), scalar=1.0,
                        in1=xT_r[:, sl].bitcast(FP), op0=OP.mult, op1=OP.mult,
                        accum_out=S_rr[:, t:t + 1],
                    )
                    sc2 = scratch.tile([128, BC], FP, tag="sq")
                    nc.vector.scalar_tensor_tensor(
                        out=sc2[:], in0=xT_i[:, sl].bitcast(FP), scalar=1.0,
                        in1=xT_i[:, sl].bitcast(FP), op0=OP.mult, op1=OP.mult,
                        accum_out=S_ii[:, t:t + 1],
                    )
                    sc3 = scratch.tile([128, BC], FP, tag="sq")
                    nc.vector.scalar_tensor_tensor(
                        out=sc3[:], in0=xT_r[:, sl].bitcast(FP), scalar=1.0,
                        in1=xT_i[:, sl].bitcast(FP), op0=OP.mult, op1=OP.mult,
                        accum_out=S_ri[:, t:t + 1],
                    )
                    # T quadratic correction for this t
                    tsl = slice(CSH * t, CSH * (t + 1))
                    ts1 = scratch.tile([128, CSH], FP, tag="tsq")
                    nc.vector.scalar_tensor_tensor(
                        out=ts1[:], in0=T_r_sb[:, tsl], scalar=1.0,
                        in1=T_r_sb[:, tsl], op0=OP.mult, op1=OP.mult,
                        accum_out=corr_rr[:, t:t + 1],
                    )
                    ts2 = scratch.tile([128, CSH], FP, tag="tsq")
                    nc.vector.scalar_tensor_tensor(
                        out=ts2[:], in0=T_i_sb[:, tsl], scalar=1.0,
                        in1=T_i_sb[:, tsl], op0=OP.mult, op1=OP.mult,
                        accum_out=corr_ii[:, t:t + 1],
                    )
                    ts3 = scratch.tile([128, CSH], FP, tag="tsq")
                    nc.vector.scalar_tensor_tensor(
                        out=ts3[:], in0=T_r_sb[:, tsl], scalar=1.0,
                        in1=T_i_sb[:, tsl], op0=OP.mult, op1=OP.mult,
                        accum_out=corr_ri[:, t:t + 1],
                    )
                # local partial covariance for this quarter's columns
                qsl = slice(4 * tg, 4 * (tg + 1))
                for j, (S, corr) in enumerate(
                    ((S_rr, corr_rr), (S_ri, corr_ri), (S_ii, corr_ii))
                ):
                    dstq = partial[:, NFT * j + 4 * tg:NFT * j + 4 * (tg + 1)]
                    nc.vector.scalar_tensor_tensor(
                        out=dstq, in0=corr[:, qsl], scalar=-1.0 / B,
                        in1=S[:, qsl], op0=OP.mult, op1=OP.add,
                    )
                    nc.vector.tensor_scalar(
                        out=dstq, in0=dstq, scalar1=1.0 / NM1, scalar2=None,
                        op0=OP.mult,
                    )

            # ---- AllGather partial covariances (24 KB in, 192 KB out), then
            # local sum of the 8 per-core partials.  AllGather avoids the
            # AllReduce's 1.875x fixed-cost multiplier.
            ar_in = dram.tile([128, 3 * NFT], FP, tag="ar_in")
            ar_out = dram.tile([NCORES * 128, 3 * NFT], FP, tag="ar_out")
            nc.sync.dma_start(ar_in[:], partial[:])
            nc.gpsimd.collective_compute(
                "AllGather", OP.bypass,
                replica_groups=[list(range(NCORES))],
                ins=[ar_in.opt()],
                outs=[ar_out.opt()],
            )
            covp = []
            for k in range(NCORES):
                cvk = scratch.tile([128, 3 * NFT], FP, tag=f"cv{k}",
                                   name=f"cv{k}")
                eng = (nc.sync, nc.scalar, nc.gpsimd)[k % 3]
                eng.dma_start(cvk[:], ar_out[128 * k:128 * (k + 1), :])
                covp.append(cvk)
            cov = small.tile([128, 3 * NFT], FP, tag="cov")
            nc.vector.tensor_tensor(out=cov[:], in0=covp[0][:], in1=covp[1][:],
                                    op=OP.add)
            for k in range(2, NCORES):
                nc.vector.tensor_tensor(out=cov[:], in0=cov[:], in1=covp[k][:],
                                        op=OP.add)

            nc.sync.dma_start(dbg[:], cov[:])

            # ---- Phase B: complex mean over F via PE ones-matmul on xT
            # (overlaps the collective -- no dependency on cov).
            _stk2 = ExitStack()
            ps_mean = _stk2.enter_context(
                tc.tile_pool(name="ps_mean", bufs=1, space="PSUM"))
            psm_r = ps_mean.tile([1, BC], FP, tag="psm_r")
            psm_i = ps_mean.tile([1, BC], FP, tag="psm_i")
            for xT, psm in ((xT_r, psm_r), (xT_i, psm_i)):
                for t in range(NFT):
                    for h in range(2):
                        nc.tensor.matmul(
                            psm[:, 512 * h:512 * (h + 1)],
                            onesF_t[:],
                            xT[:, BC * t + 512 * h:BC * t + 512 * (h + 1)],
                            start=(t == 0), stop=(t == NFT - 1),
                        )
            # M3 = [-mean_r; -mean_i; ones]  (3, 1024).  Engine ops cannot
            # write at partition offsets 1/2, so build rows at partition 0
            # and DMA them into place (3 queues in parallel).
            M3 = small.tile([3, BC], FPR, tag="M3")
            row0 = small.tile([1, BC], FPR, tag="rowtmp", name="row0")
            nc.vector.tensor_scalar(out=row0[:], in0=psm_r[:],
                                    scalar1=-1.0, scalar2=None, op0=OP.mult)
            nc.sync.dma_start(M3[0:1, :], row0[:])
            row1 = small.tile([1, BC], FPR, tag="rowtmp", name="row1")
            nc.vector.tensor_scalar(out=row1[:], in0=psm_i[:],
                                    scalar1=-1.0, scalar2=None, op0=OP.mult)
            nc.scalar.dma_start(M3[1:2, :], row1[:])
            row2 = small.tile([1, BC], FPR, tag="rowtmp", name="row2")
            nc.vector.tensor_scalar(out=row2[:], in0=psm_i[:],
                                    scalar1=0.0, scalar2=1.0, op0=OP.mult,
                                    op1=OP.add)
            nc.gpsimd.dma_start(M3[2:3, :], row2[:])
            _stk2.close()

            # ---- Phase C: closed-form 2x2 inverse sqrt, fold gamma -> A
            def stile(tag):
                return small.tile([128, NFT], FP, tag=tag, name=tag)

            arr, bri, cii = stile("arr"), stile("bri"), stile("cii")
            nc.vector.tensor_scalar(out=arr[:], in0=cov[:, 0:NFT],
                                    scalar1=EPS, scalar2=None, op0=OP.add)
            nc.vector.tensor_copy(bri[:], cov[:, NFT:2 * NFT])
            nc.vector.tensor_scalar(out=cii[:], in0=cov[:, 2 * NFT:3 * NFT],
                                    scalar1=EPS, scalar2=None, op0=OP.add)

            det, tmp = stile("det"), stile("tmp")
            nc.vector.tensor_tensor(out=det[:], in0=arr[:], in1=cii[:],
                                    op=OP.mult)
            nc.vector.tensor_tensor(out=tmp[:], in0=bri[:], in1=bri[:],
                                    op=OP.mult)
            nc.vector.tensor_tensor(out=det[:], in0=det[:], in1=tmp[:],
                                    op=OP.subtract)
            s_t = stile("s_t")
            nc.scalar.activation(s_t[:], det[:], AF.Sqrt)
            # tval = sqrt(a + c + 2 s)
            tsum = stile("tsum")
            nc.vector.tensor_tensor(out=tsum[:], in0=arr[:], in1=cii[:],
                                    op=OP.add)
            nc.vector.scalar_tensor_tensor(out=tsum[:], in0=s_t[:], scalar=2.0,
                                           in1=tsum[:], op0=OP.mult, op1=OP.add)
            tval = stile("tval")
            nc.scalar.activation(tval[:], tsum[:], AF.Sqrt)
            den, rden = stile("den"), stile("rden")
            nc.vector.tensor_tensor(out=den[:], in0=s_t[:], in1=tval[:],
                                    op=OP.mult)
            nc.vector.reciprocal(rden[:], den[:])

            w_rr, w_ii, wri_n = stile("w_rr"), stile("w_ii"), stile("wri_n")
            # w_rr = (c+s)*rden ; w_ii = (a+s)*rden ; w_ri = -b*rden = wri_n
            nc.vector.tensor_tensor(out=w_rr[:], in0=cii[:], in1=s_t[:],
                                    op=OP.add)
            nc.vector.tensor_tensor(out=w_rr[:], in0=w_rr[:], in1=rden[:],
                                    op=OP.mult)
            nc.vector.tensor_tensor(out=w_ii[:], in0=arr[:], in1=s_t[:],
                                    op=OP.add)
            nc.vector.tensor_tensor(out=w_ii[:], in0=w_ii[:], in1=rden[:],
                                    op=OP.mult)
            nc.vector.tensor_tensor(out=wri_n[:], in0=bri[:], in1=rden[:],
                                    op=OP.mult)
            nc.vector.tensor_scalar(out=wri_n[:], in0=wri_n[:], scalar1=-1.0,
                                    scalar2=None, op0=OP.mult)

            # A = G @ W,  G = [[g_r, -g_i], [g_i, g_r]], W = [[w_rr, w_ri],
            # [w_ri, w_ii]] with w_ri = wri_n
            def rtile(tag):
                return small.tile([128, NFT], FPR, tag=tag, name=tag)

            a_rr, a_ri = rtile("a_rr"), rtile("a_ri")
            a_ir, a_ii = rtile("a_ir"), rtile("a_ii")
            u, v = stile("u"), stile("v")
            # a_rr = g_r*w_rr - g_i*w_ri
            nc.vector.tensor_tensor(out=u[:], in0=g_r_t[:], in1=w_rr[:],
                                    op=OP.mult)
            nc.vector.tensor_tensor(out=v[:], in0=g_i_t[:], in1=wri_n[:],
                                    op=OP.mult)
            nc.vector.tensor_tensor(out=a_rr[:], in0=u[:], in1=v[:],
                                    op=OP.subtract)
            # a_ri = g_r*w_ri - g_i*w_ii
            nc.vector.tensor_tensor(out=u[:], in0=g_r_t[:], in1=wri_n[:],
                                    op=OP.mult)
            nc.vector.tensor_tensor(out=v[:], in0=g_i_t[:], in1=w_ii[:],
                                    op=OP.mult)
            nc.vector.tensor_tensor(out=a_ri[:], in0=u[:], in1=v[:],
                                    op=OP.subtract)
            # a_ir = g_i*w_rr + g_r*w_ri
            nc.vector.tensor_tensor(out=u[:], in0=g_i_t[:], in1=w_rr[:],
                                    op=OP.mult)
            nc.vector.tensor_tensor(out=v[:], in0=g_r_t[:], in1=wri_n[:],
                                    op=OP.mult)
            nc.vector.tensor_tensor(out=a_ir[:], in0=u[:], in1=v[:],
                                    op=OP.add)
            # a_ii = g_i*w_ri + g_r*w_ii
            nc.vector.tensor_tensor(out=u[:], in0=g_i_t[:], in1=wri_n[:],
                                    op=OP.mult)
            nc.vector.tensor_tensor(out=v[:], in0=g_r_t[:], in1=w_ii[:],
                                    op=OP.mult)
            nc.vector.tensor_tensor(out=a_ii[:], in0=u[:], in1=v[:],
                                    op=OP.add)

            # ---- A3C rhs for the K=3 correction matmul: (3, 4096)
            # row0[2f+c] = (a_rr, a_ir)[c][f]; row1: (a_ri, a_ii); row2: beta
            A3C = small.tile([3, 2 * F], FPR, tag="A3C")
            rings = (nc.sync, nc.scalar, nc.gpsimd, nc.scalar)
            for row, (ev, od) in enumerate(((a_rr, a_ir), (a_ri, a_ii))):
                for cpar, srctile in ((0, ev), (1, od)):
                    # bounce through DRAM; read back in f-major order with a
                    # strided AP.  dram layout: addr(p, t) = 16*p + t.
                    eng = rings[2 * row + cpar]
                    dbuf = dram.tile([128, NFT], FPR, tag=f"dbuf{row}{cpar}",
                                     name=f"dbuf{row}{cpar}")
                    eng.dma_start(dbuf[:], srctile[:])
                    # src iterates (t, p): steps [[1, 16], [16, 128]]
                    src = dbuf[:].rearrange("p t -> (p t)").rearrange(
                        "(p t) -> t p", p=128, t=NFT
                    )
                    dst = A3C[row:row + 1, cpar::2].rearrange(
                        "z (t p) -> z t p", t=NFT, p=128
                    )
                    eng.dma_start(dst, src)
            nc.sync.dma_start(A3C[2:3, :], beta_ilv[:])

            # release phase-A pools (xin/scratch SBUF, transpose/T PSUM)
            _stk.close()

            # ---- Phase D: apply.  t-outer; W built on the fly.  Inside each
            # PSUM group the x-matmuls come first (start=True per region) and
            # the A3C correction last, so PE needn't wait for A3C.
            _stk3 = ExitStack()
            ps_o = _stk3.enter_context(
                tc.tile_pool(name="ps_o", bufs=8, space="PSUM"))
            for t2 in range(NFT // 2):
                ta, tb = 2 * t2, 2 * t2 + 1
                Ws = []
                for t in (ta, tb):
                    W_r = wpool.tile([128, 256], FPR, tag="W_r",
                                     name=f"W_r_{t}")
                    W_i = wpool.tile([128, 256], FPR, tag="W_i",
                                     name=f"W_i_{t}")
                    for W, (ev, od) in ((W_r, (a_rr, a_ir)),
                                        (W_i, (a_ri, a_ii))):
                        Wv = W[:].rearrange("p (g c) -> p g c", c=2)
                        nc.vector.tensor_scalar(
                            out=Wv[:, :, 0], in0=ident_t[:],
                            scalar1=ev[:, t:t + 1].bitcast(FP), scalar2=None,
                            op0=OP.mult,
                        )
                        nc.scalar.activation(
                            Wv[:, :, 1], ident_t[:], AF.Copy,
                            scale=od[:, t:t + 1].bitcast(FP),
                        )
                    Ws.append((W_r, W_i))
                for bh in range(2):
                    stg = stage.tile([128, 4 * 512], FP, tag="stg")
                    pos = []
                    for bb in range(4):
                        b = 4 * bh + bb
                        po = ps_o.tile([128, 512], FP, tag="po")
                        pos.append(po)
                        nc.tensor.matmul(
                            po[:],
                            M3[:, 128 * b:128 * (b + 1)],
                            A3C[:, 512 * t2:512 * (t2 + 1)],
                            start=True, stop=False,
                        )
                        for j, t in enumerate((ta, tb)):
                            W_r, W_i = Ws[j]
                            sl = slice(BC * t + 128 * b,
                                       BC * t + 128 * (b + 1))
                            nc.tensor.matmul(
                                po[:, 256 * j:256 * (j + 1)],
                                xT_r[:, sl], W_r[:],
                                start=False, stop=False,
                            )
                            nc.tensor.matmul(
                                po[:, 256 * j:256 * (j + 1)],
                                xT_i[:, sl], W_i[:],
                                start=False, stop=(j == 1),
                            )
                    for bb in range(4):
                        po = pos[bb]
                        if (4 * bh + bb) % 2 == 0:
                            nc.vector.tensor_copy(
                                stg[:, 512 * bb:512 * (bb + 1)], po[:])
                        else:
                            nc.scalar.copy(
                                stg[:, 512 * bb:512 * (bb + 1)], po[:])
                    # 1 MB store: rows (b, p) -> out[128*b + p, 512*t2:+512]
                    dst = out.rearrange("(a p) f -> p a f", p=128)[
                        :, 4 * bh:4 * (bh + 1), 512 * t2:512 * (t2 + 1)
                    ]
                    src = stg[:].rearrange("p (a q) -> p a q", q=512)
                    seng = (nc.sync, nc.scalar, nc.sync, nc.gpsimd)[
                        (2 * t2 + bh) % 4]
                    seng.dma_start(dst, src)
            _stk3.close()

    split_multi_waits(nc)
    return nc


_CACHE = {}


def _get_nc():
    if "nc" not in _CACHE:
        _CACHE["nc"] = build_bass()
    return _CACHE["nc"]


def _constants():
    if "consts" not in _CACHE:
        sel = np.zeros((128, CSH), dtype=np.float32)
        for p in range(128):
            sel[p, p % CSH] = 1.0
        _CACHE["consts"] = {
            "ident": np.eye(128, dtype=np.float32),
            "identsel": np.ascontiguousarray(np.concatenate(
                [np.eye(128, dtype=np.float32), sel,
                 np.zeros((128, 112), dtype=np.float32)], axis=1)),
            "onesF": np.full((128, 1), 1.0 / F, dtype=np.float32),
        }
    return _CACHE["consts"]


def kernel(x_real, x_imag, gamma_r, gamma_i, beta_r, beta_i):
    x_real = np.ascontiguousarray(x_real, dtype=np.float32)
    x_imag = np.ascontiguousarray(x_imag, dtype=np.float32)
    gamma_r = np.asarray(gamma_r, dtype=np.float32)
    gamma_i = np.asarray(gamma_i, dtype=np.float32)
    beta_r = np.asarray(beta_r, dtype=np.float32)
    beta_i = np.asarray(beta_i, dtype=np.float32)

    nc = _get_nc()
    consts = _constants()
    g_r_t = np.ascontiguousarray(gamma_r.reshape(NFT, 128).T)
    g_i_t = np.ascontiguousarray(gamma_i.reshape(NFT, 128).T)
    beta_ilv = np.ascontiguousarray(
        np.stack([beta_r, beta_i], axis=-1).reshape(1, 2 * F)
    )

    in_maps = []
    for k in range(NCORES):
        cs = slice(CSH * k, CSH * (k + 1))
        in_maps.append({
            "x_r": np.ascontiguousarray(
                x_real[:, cs, :].reshape(BC, F)),
            "x_i": np.ascontiguousarray(
                x_imag[:, cs, :].reshape(BC, F)),
            "g_r": g_r_t, "g_i": g_i_t, "beta_ilv": beta_ilv,
            **consts,
        })

    res = run_bass_kernel_spmd(nc, in_maps, list(range(NCORES)))

    full = np.empty((B, C, F, 2), dtype=np.float32)
    for k in range(NCORES):
        full[:, CSH * k:CSH * (k + 1)] = (
            res.results[k]["out"].reshape(B, CSH, F, 2)
        )
    return full


# revision 18
# speedup vs baseline: 1.0603x; 1.0529x over previous
"""ComplexLayerNorm Trainium2 kernel (8 NeuronCores, SPMD, C-sharded).

Math (see reference): per-feature 2x2 covariance whitening of (re, im) over
all B*C samples (centered with the batch-only mean mu_b), after subtracting
the complex mean over F, plus complex affine.

Sharding: C (=128) split 16-per-core, so the batch-sums T[c,f] = sum_b x and
mu_b are core-local and only 3 partial second-moment F-vectors (24 KB) need a
cross-core reduction (AllGather + local sum: cheaper than AllReduce).

v2 notes (cost-model-driven):
  * All PE matmuls stream their moving operand as bf16 (exact 0/1 selector)
    or float32r (same bits as fp32, 1 cyc/row when >=256 out cols) -- 4x the
    fp32 row rate.  Stationary operands are float32r bitcast views.
  * Phase A is f-quarter-pipelined: each 512-col f-quarter is loaded,
    transposed, and its covariance stats reduced while the next quarter
    loads, so the collective fires right after the last quarter.
  * AllGather (no 1.875x AllReduce multiplier) + 7 local DVE adds.
  * A3C/M3 DRAM bounce DMAs are spread across 4 queues.
  * Phase D emits x-matmuls before the K=3 correction matmul inside each
    PSUM accumulation group so PE can start before A3C is assembled.
"""

import numpy as np

import bass_rust
import concourse.bass as bass
import concourse.mybir as mybir
from concourse import tile
from concourse.bass_utils import run_bass_kernel_spmd


def split_multi_waits(nc):
    """The walrus build in this container allows only ONE sync-wait command
    per instruction; Tile emits several.  Split extras into preceding
    single-wait NoOps on the same engine (sequential waits == AND)."""
    cnt = 0
    for bb in nc.main_func.blocks:
        il = bb.instructions
        newlist = []
        changed = False
        for inst in list(il):
            si = inst.sync_info
            waits = list(si.on_wait) if si else []
            if len(waits) > 1:
                changed = True
                for w in waits[:-1]:
                    cnt += 1
                    nop = bass_rust.InstNoOp(name=f"I-wsplit-{cnt}")
                    nop.engine = inst.engine
                    nop.sync_info = mybir.SyncInfo(on_wait=[w], on_update=[])
                    newlist.append(nop)
                inst.sync_info = mybir.SyncInfo(
                    on_wait=[waits[-1]], on_update=list(si.on_update))
            newlist.append(inst)
        if changed:
            il[:] = newlist
    return cnt

FP = mybir.dt.float32
FPR = mybir.dt.float32r
BF = mybir.dt.bfloat16
AF = mybir.ActivationFunctionType
OP = mybir.AluOpType

B, C, F = 64, 128, 2048
NCORES = 8
CSH = C // NCORES           # 16 channels per core
BC = B * CSH                # 1024 sample rows per core
NFT = F // 128              # 16 f-chunks
NBC = BC // 128             # 8 bc-chunks
NQ = 4                      # f-quarters (4 f-chunks each)
EPS = 1e-4
NM1 = float(B * C - 1)      # 8191


def r(ap):
    """float32r view of an fp32 AP (same bits; 4x faster PE streaming)."""
    return ap.bitcast(FPR)


def build_bass():
    nc = bass.Bass()

    x_r = nc.dram_tensor("x_r", [BC, F], FPR, kind="ExternalInput")
    x_i = nc.dram_tensor("x_i", [BC, F], FPR, kind="ExternalInput")
    # gamma pre-tiled on host to (128, NFT): tile[p, t] = gamma[128*t + p]
    g_r = nc.dram_tensor("g_r", [128, NFT], FP, kind="ExternalInput")
    g_i = nc.dram_tensor("g_i", [128, NFT], FP, kind="ExternalInput")
    # beta interleaved on host: (1, 4096) = [b_r[0], b_i[0], b_r[1], ...]
    beta_ilv = nc.dram_tensor("beta_ilv", [1, 2 * F], FPR, kind="ExternalInput")
    ident = nc.dram_tensor("ident", [128, 128], FP, kind="ExternalInput")
    identsel = nc.dram_tensor("identsel", [128, 256], FPR,
                              kind="ExternalInput")
    onesF = nc.dram_tensor("onesF", [128, 1], FPR, kind="ExternalInput")

    out = nc.dram_tensor("out", [BC, 2 * F], FP, kind="ExternalOutput")
    dbg = nc.dram_tensor("dbg", [128, 3 * NFT], FP, kind="ExternalOutput")

    with tile.TileContext(nc) as tc:
        with (
            tc.tile_pool(name="big", bufs=1) as big,
            tc.tile_pool(name="small", bufs=1) as small,
            tc.tile_pool(name="wpool", bufs=2) as wpool,
            tc.tile_pool(name="stage", bufs=3) as stage,
            tc.tile_pool(name="dram", bufs=1, space="DRAM") as dram,
        ):
            # ---- constants to SBUF
            ident_t = small.tile([128, 128], FP, tag="ident")
            nc.sync.dma_start(ident_t[:], ident[:])
            # [ident | sel | zero-pad] padded to 256 cols: streamed as fp32r
            # the >=256-col moving operand runs at 1 cyc/row (4x fp32), with
            # bit-exact fp32 numerics.
            identsel_t = small.tile([128, 256], FPR, tag="identsel")
            nc.sync.dma_start(identsel_t[:], identsel[:])
            onesF_t = small.tile([128, 1], FPR, tag="onesF")
            nc.sync.dma_start(onesF_t[:], onesF[:])
            g_r_t = small.tile([128, NFT], FP, tag="g_r")
            nc.sync.dma_start(g_r_t[:], g_r[:])
            g_i_t = small.tile([128, NFT], FP, tag="g_i")
            nc.sync.dma_start(g_i_t[:], g_i[:])

            # ---- persistent: x transposed, xT[p, 1024*t + j] = x[j, 128*t+p]
            xT_r = big.tile([128, NFT * BC], FPR, tag="xT_r")
            xT_i = big.tile([128, NFT * BC], FPR, tag="xT_i")

            from contextlib import ExitStack
            _stk = ExitStack()
            xin = _stk.enter_context(tc.tile_pool(name="xin", bufs=4))
            scratch = _stk.enter_context(tc.tile_pool(name="scratch", bufs=2))
            ps_xt = _stk.enter_context(
                tc.tile_pool(name="ps_xt", bufs=3, space="PSUM"))

            # T[c,f] accumulators in SBUF: [p (=f in chunk), 16*t + c].
            T_r_sb = small.tile([128, NFT * CSH], FP, tag="T_r_sb")
            T_i_sb = small.tile([128, NFT * CSH], FP, tag="T_i_sb")

            # per-f stats, packed [S_rr | S_ri | S_ii] column-chunks
            S_rr = small.tile([128, NFT], FP, tag="S_rr")
            S_ri = small.tile([128, NFT], FP, tag="S_ri")
            S_ii = small.tile([128, NFT], FP, tag="S_ii")
            corr_rr = small.tile([128, NFT], FP, tag="corr_rr")
            corr_ri = small.tile([128, NFT], FP, tag="corr_ri")
            corr_ii = small.tile([128, NFT], FP, tag="corr_ii")
            partial = small.tile([128, 3 * NFT], FP, tag="partial")

            # ---- Phase A: f-quarter-pipelined load + PE transpose + stats.
            # Quarter tg covers f-chunks 4*tg..4*tg+3 (512 f-columns).
            copy_flip = 0
            for tg in range(NQ):
                fsl = slice(512 * tg, 512 * (tg + 1))
                for b in range(NBC):
                    rsl = slice(128 * b, 128 * (b + 1))
                    xn_r = xin.tile([128, 512], FPR, tag="xn")
                    nc.sync.dma_start(xn_r[:], x_r[rsl, fsl])
                    xn_i = xin.tile([128, 512], FPR, tag="xn")
                    nc.scalar.dma_start(xn_i[:], x_i[rsl, fsl])
                    for xn, xT, T_sb in (
                        (xn_r, xT_r, T_r_sb), (xn_i, xT_i, T_i_sb)
                    ):
                        # one matmul per f-chunk: rhs [ident | sel] gives the
                        # 128-col transpose AND the 16-col T partial. regions
                        # at 256-col spacing so no mm output crosses a bank.
                        pxt = ps_xt.tile([128, 1024], FP, tag="pxt")
                        for tt in range(4):
                            t = 4 * tg + tt
                            nc.tensor.matmul(
                                pxt[:, 256 * tt:256 * (tt + 1)],
                                xn[:, 128 * tt:128 * (tt + 1)],
                                identsel_t[:],
                                start=True, stop=True,
                            )
                        pv = pxt[:].rearrange("p (a q) -> p a q", q=256)
                        dst = xT[:].rearrange("p (a q) -> p a q", q=1024)[
                            :, 4 * tg:4 * (tg + 1), 128 * b:128 * (b + 1)
                        ]
                        nc.scalar.copy(dst, pv[:, :, 0:128])
                        copy_flip += 1
                        tdst = T_sb[:, 64 * tg:64 * (tg + 1)].rearrange(
                            "p (a q) -> p a q", q=CSH)
                        tsrc = pv[:, :, 128:128 + CSH]
                        if b == 0:
                            nc.vector.tensor_copy(tdst, tsrc)
                        else:
                            nc.vector.scalar_tensor_tensor(
                                out=tdst, in0=tsrc, scalar=1.0, in1=tdst,
                                op0=OP.mult, op1=OP.add,
                            )

                # quarter's xT rows are complete: second moments for its 4 t's
                for tt in range(4):
                    t = 4 * tg + tt
                    sl = slice(BC * t, BC * (t + 1))
                    sc1 = scratch.tile([128, BC], FP, tag="sq")
                    nc.vector.scalar_tensor_tensor(
                        out=sc1[:], in0=xT_r[:, sl].bitcast(FP), scalar=1.0,
                        in1=xT_r[:, sl].bitcast(FP), op0=OP.mult, op1=OP.mult,
                        accum_out=S_rr[:, t:t + 1],
                    )
                    sc2 = scratch.tile([128, BC], FP, tag="sq")
                    nc.vector.scalar_tensor_tensor(
                        out=sc2[:], in0=xT_i[:, sl].bitcast(FP), scalar=1.0,
                        in1=xT_i[:, sl].bitcast(FP), op0=OP.mult, op1=OP.mult,
                        accum_out=S_ii[:, t:t + 1],
                    )
                    sc3 = scratch.tile([128, BC], FP, tag="sq")
                    nc.vector.scalar_tensor_tensor(
                        out=sc3[:], in0=xT_r[:, sl].bitcast(FP), scalar=1.0,
                        in1=xT_i[:, sl].bitcast(FP), op0=OP.mult, op1=OP.mult,
                        accum_out=S_ri[:, t:t + 1],
                    )
                    # T quadratic correction for this t
                    tsl = slice(CSH * t, CSH * (t + 1))
                    ts1 = scratch.tile([128, CSH], FP, tag="tsq")
                    nc.vector.scalar_tensor_tensor(
                        out=ts1[:], in0=T_r_sb[:, tsl], scalar=1.0,
                        in1=T_r_sb[:, tsl], op0=OP.mult, op1=OP.mult,
                        accum_out=corr_rr[:, t:t + 1],
                    )
                    ts2 = scratch.tile([128, CSH], FP, tag="tsq")
                    nc.vector.scalar_tensor_tensor(
                        out=ts2[:], in0=T_i_sb[:, tsl], scalar=1.0,
                        in1=T_i_sb[:, tsl], op0=OP.mult, op1=OP.mult,
                        accum_out=corr_ii[:, t:t + 1],
                    )
                    ts3 = scratch.tile([128, CSH], FP, tag="tsq")
                    nc.vector.scalar_tensor_tensor(
                        out=ts3[:], in0=T_r_sb[:, tsl], scalar=1.0,
                        in1=T_i_sb[:, tsl], op0=OP.mult, op1=OP.mult,
                        accum_out=corr_ri[:, t:t + 1],
                    )
                # local partial covariance for this quarter's columns
                qsl = slice(4 * tg, 4 * (tg + 1))
                for j, (S, corr) in enumerate(
                    ((S_rr, corr_rr), (S_ri, corr_ri), (S_ii, corr_ii))
                ):
                    dstq = partial[:, NFT * j + 4 * tg:NFT * j + 4 * (tg + 1)]
                    nc.vector.scalar_tensor_tensor(
                        out=dstq, in0=corr[:, qsl], scalar=-1.0 / B,
                        in1=S[:, qsl], op0=OP.mult, op1=OP.add,
                    )
                    nc.vector.tensor_scalar(
                        out=dstq, in0=dstq, scalar1=1.0 / NM1, scalar2=None,
                        op0=OP.mult,
                    )

            # release phase-A pools (xin/scratch SBUF, transpose/T PSUM)
            _stk.close()

            # ---- AllGather partial covariances (24 KB in, 192 KB out), then
            # local sum of the 8 per-core partials.  AllGather avoids the
            # AllReduce's 1.875x fixed-cost multiplier.
            ar_in = dram.tile([128, 3 * NFT], FP, tag="ar_in")
            ar_out = dram.tile([NCORES * 128, 3 * NFT], FP, tag="ar_out")
            nc.sync.dma_start(ar_in[:], partial[:])
            nc.gpsimd.collective_compute(
                "AllGather", OP.bypass,
                replica_groups=[list(range(NCORES))],
                ins=[ar_in.opt()],
                outs=[ar_out.opt()],
            )
            covp = []
            for k in range(NCORES):
                cvk = small.tile([128, 3 * NFT], FP, tag=f"cv{k}",
                                   name=f"cv{k}")
                eng = (nc.sync, nc.scalar, nc.gpsimd)[k % 3]
                eng.dma_start(cvk[:], ar_out[128 * k:128 * (k + 1), :])
                covp.append(cvk)
            cov = small.tile([128, 3 * NFT], FP, tag="cov")
            nc.vector.tensor_tensor(out=cov[:], in0=covp[0][:], in1=covp[1][:],
                                    op=OP.add)
            for k in range(2, NCORES):
                nc.vector.tensor_tensor(out=cov[:], in0=cov[:], in1=covp[k][:],
                                        op=OP.add)

            nc.sync.dma_start(dbg[:], cov[:])

            # ---- Phase B: complex mean over F via PE ones-matmul on xT
            # (overlaps the collective -- no dependency on cov).
            _stk2 = ExitStack()
            ps_mean = _stk2.enter_context(
                tc.tile_pool(name="ps_mean", bufs=1, space="PSUM"))
            psm_r = ps_mean.tile([1, BC], FP, tag="psm_r")
            psm_i = ps_mean.tile([1, BC], FP, tag="psm_i")
            for xT, psm in ((xT_r, psm_r), (xT_i, psm_i)):
                for t in range(NFT):
                    for h in range(2):
                        nc.tensor.matmul(
                            psm[:, 512 * h:512 * (h + 1)],
                            onesF_t[:],
                            xT[:, BC * t + 512 * h:BC * t + 512 * (h + 1)],
                            start=(t == 0), stop=(t == NFT - 1),
                        )
            # M3 = [-mean_r; -mean_i; ones]  (3, 1024).  Engine ops cannot
            # write at partition offsets 1/2, so build rows at partition 0
            # and DMA them into place (3 queues in parallel).
            M3 = small.tile([3, BC], FPR, tag="M3")
            row0 = small.tile([1, BC], FPR, tag="rowtmp", name="row0")
            nc.vector.tensor_scalar(out=row0[:], in0=psm_r[:],
                                    scalar1=-1.0, scalar2=None, op0=OP.mult)
            nc.sync.dma_start(M3[0:1, :], row0[:])
            row1 = small.tile([1, BC], FPR, tag="rowtmp", name="row1")
            nc.vector.tensor_scalar(out=row1[:], in0=psm_i[:],
                                    scalar1=-1.0, scalar2=None, op0=OP.mult)
            nc.scalar.dma_start(M3[1:2, :], row1[:])
            row2 = small.tile([1, BC], FPR, tag="rowtmp", name="row2")
            nc.vector.tensor_scalar(out=row2[:], in0=psm_i[:],
                                    scalar1=0.0, scalar2=1.0, op0=OP.mult,
                                    op1=OP.add)
            nc.gpsimd.dma_start(M3[2:3, :], row2[:])
            _stk2.close()

            # ---- Phase C: closed-form 2x2 inverse sqrt, fold gamma -> A
            def stile(tag):
                return small.tile([128, NFT], FP, tag=tag, name=tag)

            arr, bri, cii = stile("arr"), stile("bri"), stile("cii")
            nc.vector.tensor_scalar(out=arr[:], in0=cov[:, 0:NFT],
                                    scalar1=EPS, scalar2=None, op0=OP.add)
            nc.vector.tensor_copy(bri[:], cov[:, NFT:2 * NFT])
            nc.vector.tensor_scalar(out=cii[:], in0=cov[:, 2 * NFT:3 * NFT],
                                    scalar1=EPS, scalar2=None, op0=OP.add)

            det, tmp = stile("det"), stile("tmp")
            nc.vector.tensor_tensor(out=det[:], in0=arr[:], in1=cii[:],
                                    op=OP.mult)
            nc.vector.tensor_tensor(out=tmp[:], in0=bri[:], in1=bri[:],
                                    op=OP.mult)
            nc.vector.tensor_tensor(out=det[:], in0=det[:], in1=tmp[:],
                                    op=OP.subtract)
            s_t = stile("s_t")
            nc.scalar.activation(s_t[:], det[:], AF.Sqrt)
            # tval = sqrt(a + c + 2 s)
            tsum = stile("tsum")
            nc.vector.tensor_tensor(out=tsum[:], in0=arr[:], in1=cii[:],
                                    op=OP.add)
            nc.vector.scalar_tensor_tensor(out=tsum[:], in0=s_t[:], scalar=2.0,
                                           in1=tsum[:], op0=OP.mult, op1=OP.add)
            tval = stile("tval")
            nc.scalar.activation(tval[:], tsum[:], AF.Sqrt)
            den, rden = stile("den"), stile("rden")
            nc.vector.tensor_tensor(out=den[:], in0=s_t[:], in1=tval[:],
                                    op=OP.mult)
            nc.vector.reciprocal(rden[:], den[:])

            w_rr, w_ii, wri_n = stile("w_rr"), stile("w_ii"), stile("wri_n")
            # w_rr = (c+s)*rden ; w_ii = (a+s)*rden ; w_ri = -b*rden = wri_n
            nc.vector.tensor_tensor(out=w_rr[:], in0=cii[:], in1=s_t[:],
                                    op=OP.add)
            nc.vector.tensor_tensor(out=w_rr[:], in0=w_rr[:], in1=rden[:],
                                    op=OP.mult)
            nc.vector.tensor_tensor(out=w_ii[:], in0=arr[:], in1=s_t[:],
                                    op=OP.add)
            nc.vector.tensor_tensor(out=w_ii[:], in0=w_ii[:], in1=rden[:],
                                    op=OP.mult)
            nc.vector.tensor_tensor(out=wri_n[:], in0=bri[:], in1=rden[:],
                                    op=OP.mult)
            nc.vector.tensor_scalar(out=wri_n[:], in0=wri_n[:], scalar1=-1.0,
                                    scalar2=None, op0=OP.mult)

            # A = G @ W,  G = [[g_r, -g_i], [g_i, g_r]], W = [[w_rr, w_ri],
            # [w_ri, w_ii]] with w_ri = wri_n
            def rtile(tag):
                return small.tile([128, NFT], FPR, tag=tag, name=tag)

            a_rr, a_ri = rtile("a_rr"), rtile("a_ri")
            a_ir, a_ii = rtile("a_ir"), rtile("a_ii")
            u, v = stile("u"), stile("v")
            # a_rr = g_r*w_rr - g_i*w_ri
            nc.vector.tensor_tensor(out=u[:], in0=g_r_t[:], in1=w_rr[:],
                                    op=OP.mult)
            nc.vector.tensor_tensor(out=v[:], in0=g_i_t[:], in1=wri_n[:],
                                    op=OP.mult)
            nc.vector.tensor_tensor(out=a_rr[:], in0=u[:], in1=v[:],
                                    op=OP.subtract)
            # a_ri = g_r*w_ri - g_i*w_ii
            nc.vector.tensor_tensor(out=u[:], in0=g_r_t[:], in1=wri_n[:],
                                    op=OP.mult)
            nc.vector.tensor_tensor(out=v[:], in0=g_i_t[:], in1=w_ii[:],
                                    op=OP.mult)
            nc.vector.tensor_tensor(out=a_ri[:], in0=u[:], in1=v[:],
                                    op=OP.subtract)
            # a_ir = g_i*w_rr + g_r*w_ri
            nc.vector.tensor_tensor(out=u[:], in0=g_i_t[:], in1=w_rr[:],
                                    op=OP.mult)
            nc.vector.tensor_tensor(out=v[:], in0=g_r_t[:], in1=wri_n[:],
                                    op=OP.mult)
            nc.vector.tensor_tensor(out=a_ir[:], in0=u[:], in1=v[:],
                                    op=OP.add)
            # a_ii = g_i*w_ri + g_r*w_ii
            nc.vector.tensor_tensor(out=u[:], in0=g_i_t[:], in1=wri_n[:],
                                    op=OP.mult)
            nc.vector.tensor_tensor(out=v[:], in0=g_r_t[:], in1=w_ii[:],
                                    op=OP.mult)
            nc.vector.tensor_tensor(out=a_ii[:], in0=u[:], in1=v[:],
                                    op=OP.add)

            # ---- A3C rhs for the K=3 correction matmul: (3, 4096)
            # row0[2f+c] = (a_rr, a_ir)[c][f]; row1: (a_ri, a_ii); row2: beta
            A3C = small.tile([3, 2 * F], FPR, tag="A3C")
            rings = (nc.sync, nc.scalar, nc.gpsimd, nc.scalar)
            for row, (ev, od) in enumerate(((a_rr, a_ir), (a_ri, a_ii))):
                for cpar, srctile in ((0, ev), (1, od)):
                    # bounce through DRAM; read back in f-major order with a
                    # strided AP.  dram layout: addr(p, t) = 16*p + t.
                    eng = rings[2 * row + cpar]
                    dbuf = dram.tile([128, NFT], FPR, tag=f"dbuf{row}{cpar}",
                                     name=f"dbuf{row}{cpar}")
                    eng.dma_start(dbuf[:], srctile[:])
                    # src iterates (t, p): steps [[1, 16], [16, 128]]
                    src = dbuf[:].rearrange("p t -> (p t)").rearrange(
                        "(p t) -> t p", p=128, t=NFT
                    )
                    dst = A3C[row:row + 1, cpar::2].rearrange(
                        "z (t p) -> z t p", t=NFT, p=128
                    )
                    eng.dma_start(dst, src)
            nc.sync.dma_start(A3C[2:3, :], beta_ilv[:])

            # ---- Phase D: apply.  t-outer; W built on the fly.  Inside each
            # PSUM group the x-matmuls come first (start=True per region) and
            # the A3C correction last, so PE needn't wait for A3C.
            _stk3 = ExitStack()
            ps_o = _stk3.enter_context(
                tc.tile_pool(name="ps_o", bufs=8, space="PSUM"))
            for t2 in range(NFT // 2):
                ta, tb = 2 * t2, 2 * t2 + 1
                Ws = []
                for t in (ta, tb):
                    W_r = wpool.tile([128, 256], FPR, tag="W_r",
                                     name=f"W_r_{t}")
                    W_i = wpool.tile([128, 256], FPR, tag="W_i",
                                     name=f"W_i_{t}")
                    for W, (ev, od) in ((W_r, (a_rr, a_ir)),
                                        (W_i, (a_ri, a_ii))):
                        Wv = W[:].rearrange("p (g c) -> p g c", c=2)
                        nc.vector.tensor_scalar(
                            out=Wv[:, :, 0], in0=ident_t[:],
                            scalar1=ev[:, t:t + 1].bitcast(FP), scalar2=None,
                            op0=OP.mult,
                        )
                        nc.scalar.activation(
                            Wv[:, :, 1], ident_t[:], AF.Copy,
                            scale=od[:, t:t + 1].bitcast(FP),
                        )
                    Ws.append((W_r, W_i))
                for bh in range(2):
                    stg = stage.tile([128, 4 * 512], FP, tag="stg")
                    pos = []
                    for bb in range(4):
                        b = 4 * bh + bb
                        po = ps_o.tile([128, 512], FP, tag="po")
                        pos.append(po)
                        nc.tensor.matmul(
                            po[:],
                            M3[:, 128 * b:128 * (b + 1)],
                            A3C[:, 512 * t2:512 * (t2 + 1)],
                            start=True, stop=False,
                        )
                        for j, t in enumerate((ta, tb)):
                            W_r, W_i = Ws[j]
                            sl = slice(BC * t + 128 * b,
                                       BC * t + 128 * (b + 1))
                            nc.tensor.matmul(
                                po[:, 256 * j:256 * (j + 1)],
                                xT_r[:, sl], W_r[:],
                                start=False, stop=False,
                            )
                            nc.tensor.matmul(
                                po[:, 256 * j:256 * (j + 1)],
                                xT_i[:, sl], W_i[:],
                                start=False, stop=(j == 1),
                            )
                    for bb in range(4):
                        po = pos[bb]
                        if (4 * bh + bb) % 2 == 0:
                            nc.vector.tensor_copy(
                                stg[:, 512 * bb:512 * (bb + 1)], po[:])
                        else:
                            nc.scalar.copy(
                                stg[:, 512 * bb:512 * (bb + 1)], po[:])
                    # 1 MB store: rows (b, p) -> out[128*b + p, 512*t2:+512]
                    dst = out.rearrange("(a p) f -> p a f", p=128)[
                        :, 4 * bh:4 * (bh + 1), 512 * t2:512 * (t2 + 1)
                    ]
                    src = stg[:].rearrange("p (a q) -> p a q", q=512)
                    seng = (nc.sync, nc.scalar, nc.sync, nc.gpsimd)[
                        (2 * t2 + bh) % 4]
                    seng.dma_start(dst, src)
            _stk3.close()

    split_multi_waits(nc)
    return nc


_CACHE = {}


def _get_nc():
    if "nc" not in _CACHE:
        _CACHE["nc"] = build_bass()
    return _CACHE["nc"]


def _constants():
    if "consts" not in _CACHE:
        sel = np.zeros((128, CSH), dtype=np.float32)
        for p in range(128):
            sel[p, p % CSH] = 1.0
        _CACHE["consts"] = {
            "ident": np.eye(128, dtype=np.float32),
            "identsel": np.ascontiguousarray(np.concatenate(
                [np.eye(128, dtype=np.float32), sel,
                 np.zeros((128, 112), dtype=np.float32)], axis=1)),
            "onesF": np.full((128, 1), 1.0 / F, dtype=np.float32),
        }
    return _CACHE["consts"]


def kernel(x_real, x_imag, gamma_r, gamma_i, beta_r, beta_i):
    x_real = np.ascontiguousarray(x_real, dtype=np.float32)
    x_imag = np.ascontiguousarray(x_imag, dtype=np.float32)
    gamma_r = np.asarray(gamma_r, dtype=np.float32)
    gamma_i = np.asarray(gamma_i, dtype=np.float32)
    beta_r = np.asarray(beta_r, dtype=np.float32)
    beta_i = np.asarray(beta_i, dtype=np.float32)

    nc = _get_nc()
    consts = _constants()
    g_r_t = np.ascontiguousarray(gamma_r.reshape(NFT, 128).T)
    g_i_t = np.ascontiguousarray(gamma_i.reshape(NFT, 128).T)
    beta_ilv = np.ascontiguousarray(
        np.stack([beta_r, beta_i], axis=-1).reshape(1, 2 * F)
    )

    in_maps = []
    for k in range(NCORES):
        cs = slice(CSH * k, CSH * (k + 1))
        in_maps.append({
            "x_r": np.ascontiguousarray(
                x_real[:, cs, :].reshape(BC, F)),
            "x_i": np.ascontiguousarray(
                x_imag[:, cs, :].reshape(BC, F)),
            "g_r": g_r_t, "g_i": g_i_t, "beta_ilv": beta_ilv,
            **consts,
        })

    res = run_bass_kernel_spmd(nc, in_maps, list(range(NCORES)))

    full = np.empty((B, C, F, 2), dtype=np.float32)
    for k in range(NCORES):
        full[:, CSH * k:CSH * (k + 1)] = (
            res.results[k]["out"].reshape(B, CSH, F, 2)
        )
    return full


# revision 19
# speedup vs baseline: 1.1221x; 1.0583x over previous
"""ComplexLayerNorm Trainium2 kernel (8 NeuronCores, SPMD, C-sharded).

Math (see reference): per-feature 2x2 covariance whitening of (re, im) over
all B*C samples (centered with the batch-only mean mu_b), after subtracting
the complex mean over F, plus complex affine.

Sharding: C (=128) split 16-per-core, so the batch-sums T[c,f] = sum_b x and
mu_b are core-local and only 3 partial second-moment F-vectors (24 KB) need a
cross-core reduction (AllGather + local sum: cheaper than AllReduce).

v2 notes (cost-model-driven):
  * All PE matmuls stream their moving operand as bf16 (exact 0/1 selector)
    or float32r (same bits as fp32, 1 cyc/row when >=256 out cols) -- 4x the
    fp32 row rate.  Stationary operands are float32r bitcast views.
  * Phase A is f-quarter-pipelined: each 512-col f-quarter is loaded,
    transposed, and its covariance stats reduced while the next quarter
    loads, so the collective fires right after the last quarter.
  * AllGather (no 1.875x AllReduce multiplier) + 7 local DVE adds.
  * A3C/M3 DRAM bounce DMAs are spread across 4 queues.
  * Phase D emits x-matmuls before the K=3 correction matmul inside each
    PSUM accumulation group so PE can start before A3C is assembled.
"""

import numpy as np

import bass_rust
import concourse.bass as bass
import concourse.mybir as mybir
from concourse import tile
from concourse.bass_utils import run_bass_kernel_spmd


def split_multi_waits(nc):
    """The walrus build in this container allows only ONE sync-wait command
    per instruction; Tile emits several.  Split extras into preceding
    single-wait NoOps on the same engine (sequential waits == AND)."""
    cnt = 0
    for bb in nc.main_func.blocks:
        il = bb.instructions
        newlist = []
        changed = False
        for inst in list(il):
            si = inst.sync_info
            waits = list(si.on_wait) if si else []
            if len(waits) > 1:
                changed = True
                for w in waits[:-1]:
                    cnt += 1
                    nop = bass_rust.InstNoOp(name=f"I-wsplit-{cnt}")
                    nop.engine = inst.engine
                    nop.sync_info = mybir.SyncInfo(on_wait=[w], on_update=[])
                    newlist.append(nop)
                inst.sync_info = mybir.SyncInfo(
                    on_wait=[waits[-1]], on_update=list(si.on_update))
            newlist.append(inst)
        if changed:
            il[:] = newlist
    return cnt

FP = mybir.dt.float32
FPR = mybir.dt.float32r
BF = mybir.dt.bfloat16
AF = mybir.ActivationFunctionType
OP = mybir.AluOpType

B, C, F = 64, 128, 2048
NCORES = 8
CSH = C // NCORES           # 16 channels per core
BC = B * CSH                # 1024 sample rows per core
NFT = F // 128              # 16 f-chunks
NBC = BC // 128             # 8 bc-chunks
NQ = 4                      # f-quarters (4 f-chunks each)
EPS = 1e-4
NM1 = float(B * C - 1)      # 8191


def r(ap):
    """float32r view of an fp32 AP (same bits; 4x faster PE streaming)."""
    return ap.bitcast(FPR)


def build_bass():
    nc = bass.Bass()

    x_r = nc.dram_tensor("x_r", [BC, F], FPR, kind="ExternalInput")
    x_i = nc.dram_tensor("x_i", [BC, F], FPR, kind="ExternalInput")
    # gamma pre-tiled on host to (128, NFT): tile[p, t] = gamma[128*t + p]
    g_r = nc.dram_tensor("g_r", [128, NFT], FP, kind="ExternalInput")
    g_i = nc.dram_tensor("g_i", [128, NFT], FP, kind="ExternalInput")
    # beta interleaved on host: (1, 4096) = [b_r[0], b_i[0], b_r[1], ...]
    beta_ilv = nc.dram_tensor("beta_ilv", [1, 2 * F], FPR, kind="ExternalInput")
    ident = nc.dram_tensor("ident", [128, 128], FP, kind="ExternalInput")
    identsel = nc.dram_tensor("identsel", [128, 256], FPR,
                              kind="ExternalInput")
    onesF = nc.dram_tensor("onesF", [128, 1], FP, kind="ExternalInput")

    out = nc.dram_tensor("out", [BC, 2 * F], FP, kind="ExternalOutput")

    with tile.TileContext(nc) as tc:
        with (
            tc.tile_pool(name="big", bufs=1) as big,
            tc.tile_pool(name="small", bufs=1) as small,
            tc.tile_pool(name="wpool", bufs=2) as wpool,
            tc.tile_pool(name="stage", bufs=4) as stage,
            tc.tile_pool(name="dram", bufs=1, space="DRAM") as dram,
        ):
            # ---- constants to SBUF
            ident_t = small.tile([128, 128], FP, tag="ident")
            nc.sync.dma_start(ident_t[:], ident[:])
            # [ident | sel | zero-pad] padded to 256 cols: streamed as fp32r
            # the >=256-col moving operand runs at 1 cyc/row (4x fp32), with
            # bit-exact fp32 numerics.
            identsel_t = small.tile([128, 256], FPR, tag="identsel")
            nc.sync.dma_start(identsel_t[:], identsel[:])
            onesF_t = small.tile([128, 1], FP, tag="onesF")
            nc.sync.dma_start(onesF_t[:], onesF[:])
            onesF_b = small.tile([128, 1], BF, tag="onesF_b")
            nc.vector.tensor_copy(onesF_b[:], onesF_t[:])
            g_r_t = small.tile([128, NFT], FP, tag="g_r")
            nc.sync.dma_start(g_r_t[:], g_r[:])
            g_i_t = small.tile([128, NFT], FP, tag="g_i")
            nc.sync.dma_start(g_i_t[:], g_i[:])

            # ---- persistent: x transposed, xT[p, 1024*t + j] = x[j, 128*t+p]
            xT_r = big.tile([128, NFT * BC], BF, tag="xT_r")
            xT_i = big.tile([128, NFT * BC], BF, tag="xT_i")

            from contextlib import ExitStack
            _stk = ExitStack()
            xin = _stk.enter_context(tc.tile_pool(name="xin", bufs=8))
            scratch = _stk.enter_context(tc.tile_pool(name="scratch", bufs=3))
            ps_xt = _stk.enter_context(
                tc.tile_pool(name="ps_xt", bufs=4, space="PSUM"))

            # T[c,f] accumulators in SBUF: [p (=f in chunk), 16*t + c].
            T_r_sb = small.tile([128, NFT * CSH], FP, tag="T_r_sb")
            T_i_sb = small.tile([128, NFT * CSH], FP, tag="T_i_sb")

            # per-f stats, packed [S_rr | S_ri | S_ii] column-chunks
            S_rr = small.tile([128, NFT], FP, tag="S_rr")
            S_ri = small.tile([128, NFT], FP, tag="S_ri")
            S_ii = small.tile([128, NFT], FP, tag="S_ii")
            corr_rr = small.tile([128, NFT], FP, tag="corr_rr")
            corr_ri = small.tile([128, NFT], FP, tag="corr_ri")
            corr_ii = small.tile([128, NFT], FP, tag="corr_ii")
            partial = small.tile([128, 3 * NFT], FP, tag="partial")

            # ---- Phase A: f-quarter-pipelined load + PE transpose + stats.
            # Quarter tg covers f-chunks 4*tg..4*tg+3 (512 f-columns).
            copy_flip = 0
            for tg in range(NQ):
                fsl = slice(512 * tg, 512 * (tg + 1))
                for b in range(NBC):
                    rsl = slice(128 * b, 128 * (b + 1))
                    xn_r = xin.tile([128, 512], FPR, tag="xn")
                    nc.sync.dma_start(xn_r[:], x_r[rsl, fsl])
                    xn_i = xin.tile([128, 512], FPR, tag="xn")
                    nc.scalar.dma_start(xn_i[:], x_i[rsl, fsl])
                    for xn, xT, T_sb in (
                        (xn_r, xT_r, T_r_sb), (xn_i, xT_i, T_i_sb)
                    ):
                        # one matmul per f-chunk: rhs [ident | sel] gives the
                        # 128-col transpose AND the 16-col T partial. regions
                        # at 256-col spacing so no mm output crosses a bank.
                        pxt = ps_xt.tile([128, 1024], FP, tag="pxt")
                        for tt in range(4):
                            t = 4 * tg + tt
                            nc.tensor.matmul(
                                pxt[:, 256 * tt:256 * (tt + 1)],
                                xn[:, 128 * tt:128 * (tt + 1)],
                                identsel_t[:],
                                start=True, stop=True,
                            )
                        pv = pxt[:].rearrange("p (a q) -> p a q", q=256)
                        dst = xT[:].rearrange("p (a q) -> p a q", q=1024)[
                            :, 4 * tg:4 * (tg + 1), 128 * b:128 * (b + 1)
                        ]
                        nc.scalar.copy(dst, pv[:, :, 0:128])
                        copy_flip += 1
                        tdst = T_sb[:, 64 * tg:64 * (tg + 1)].rearrange(
                            "p (a q) -> p a q", q=CSH)
                        tsrc = pv[:, :, 128:128 + CSH]
                        if b == 0:
                            nc.vector.tensor_copy(tdst, tsrc)
                        else:
                            nc.vector.scalar_tensor_tensor(
                                out=tdst, in0=tsrc, scalar=1.0, in1=tdst,
                                op0=OP.mult, op1=OP.add,
                            )

                # quarter's xT rows are complete: second moments for its 4 t's
                for tt in range(4):
                    t = 4 * tg + tt
                    sl = slice(BC * t, BC * (t + 1))
                    sc1 = scratch.tile([128, BC], BF, tag="sq")
                    nc.vector.scalar_tensor_tensor(
                        out=sc1[:], in0=xT_r[:, sl], scalar=1.0,
                        in1=xT_r[:, sl], op0=OP.mult, op1=OP.mult,
                        accum_out=S_rr[:, t:t + 1],
                    )
                    sc2 = scratch.tile([128, BC], BF, tag="sq")
                    nc.vector.scalar_tensor_tensor(
                        out=sc2[:], in0=xT_i[:, sl], scalar=1.0,
                        in1=xT_i[:, sl], op0=OP.mult, op1=OP.mult,
                        accum_out=S_ii[:, t:t + 1],
                    )
                    sc3 = scratch.tile([128, BC], BF, tag="sq")
                    nc.vector.scalar_tensor_tensor(
                        out=sc3[:], in0=xT_r[:, sl], scalar=1.0,
                        in1=xT_i[:, sl], op0=OP.mult, op1=OP.mult,
                        accum_out=S_ri[:, t:t + 1],
                    )
                    # T quadratic correction for this t
                    tsl = slice(CSH * t, CSH * (t + 1))
                    ts1 = scratch.tile([128, CSH], FP, tag="tsq")
                    nc.vector.scalar_tensor_tensor(
                        out=ts1[:], in0=T_r_sb[:, tsl], scalar=1.0,
                        in1=T_r_sb[:, tsl], op0=OP.mult, op1=OP.mult,
                        accum_out=corr_rr[:, t:t + 1],
                    )
                    ts2 = scratch.tile([128, CSH], FP, tag="tsq")
                    nc.vector.scalar_tensor_tensor(
                        out=ts2[:], in0=T_i_sb[:, tsl], scalar=1.0,
                        in1=T_i_sb[:, tsl], op0=OP.mult, op1=OP.mult,
                        accum_out=corr_ii[:, t:t + 1],
                    )
                    ts3 = scratch.tile([128, CSH], FP, tag="tsq")
                    nc.vector.scalar_tensor_tensor(
                        out=ts3[:], in0=T_r_sb[:, tsl], scalar=1.0,
                        in1=T_i_sb[:, tsl], op0=OP.mult, op1=OP.mult,
                        accum_out=corr_ri[:, t:t + 1],
                    )
                # local partial covariance for this quarter's columns
                qsl = slice(4 * tg, 4 * (tg + 1))
                for j, (S, corr) in enumerate(
                    ((S_rr, corr_rr), (S_ri, corr_ri), (S_ii, corr_ii))
                ):
                    dstq = partial[:, NFT * j + 4 * tg:NFT * j + 4 * (tg + 1)]
                    nc.vector.scalar_tensor_tensor(
                        out=dstq, in0=corr[:, qsl], scalar=-1.0 / B,
                        in1=S[:, qsl], op0=OP.mult, op1=OP.add,
                    )
                    nc.vector.tensor_scalar(
                        out=dstq, in0=dstq, scalar1=1.0 / NM1, scalar2=None,
                        op0=OP.mult,
                    )

            # release phase-A pools (xin/scratch SBUF, transpose/T PSUM)
            _stk.close()

            # ---- AllGather partial covariances (24 KB in, 192 KB out), then
            # local sum of the 8 per-core partials.  AllGather avoids the
            # AllReduce's 1.875x fixed-cost multiplier.
            ar_in = dram.tile([128, 3 * NFT], FP, tag="ar_in")
            ar_out = dram.tile([NCORES * 128, 3 * NFT], FP, tag="ar_out")
            nc.sync.dma_start(ar_in[:], partial[:])
            nc.gpsimd.collective_compute(
                "AllGather", OP.bypass,
                replica_groups=[list(range(NCORES))],
                ins=[ar_in.opt()],
                outs=[ar_out.opt()],
            )
            covp = []
            for k in range(NCORES):
                cvk = small.tile([128, 3 * NFT], FP, tag=f"cv{k}",
                                   name=f"cv{k}")
                eng = (nc.sync, nc.scalar, nc.gpsimd)[k % 3]
                eng.dma_start(cvk[:], ar_out[128 * k:128 * (k + 1), :])
                covp.append(cvk)
            cov = small.tile([128, 3 * NFT], FP, tag="cov")
            nc.vector.tensor_tensor(out=cov[:], in0=covp[0][:], in1=covp[1][:],
                                    op=OP.add)
            for k in range(2, NCORES):
                nc.vector.tensor_tensor(out=cov[:], in0=cov[:], in1=covp[k][:],
                                        op=OP.add)

            # ---- Phase B: complex mean over F via PE ones-matmul on xT
            # (overlaps the collective -- no dependency on cov).
            _stk2 = ExitStack()
            ps_mean = _stk2.enter_context(
                tc.tile_pool(name="ps_mean", bufs=1, space="PSUM"))
            psm_r = ps_mean.tile([1, BC], FP, tag="psm_r")
            psm_i = ps_mean.tile([1, BC], FP, tag="psm_i")
            for xT, psm in ((xT_r, psm_r), (xT_i, psm_i)):
                for t in range(NFT):
                    for h in range(2):
                        nc.tensor.matmul(
                            psm[:, 512 * h:512 * (h + 1)],
                            onesF_b[:],
                            xT[:, BC * t + 512 * h:BC * t + 512 * (h + 1)],
                            start=(t == 0), stop=(t == NFT - 1),
                        )
            # M3 = [-mean_r; -mean_i; ones]  (3, 1024).  Engine ops cannot
            # write at partition offsets 1/2, so build rows at partition 0
            # and DMA them into place (3 queues in parallel).
            M3 = small.tile([3, BC], FPR, tag="M3")
            row0 = small.tile([1, BC], FPR, tag="rowtmp")
            nc.vector.tensor_scalar(out=row0[:], in0=psm_r[:],
                                    scalar1=-1.0, scalar2=None, op0=OP.mult)
            nc.sync.dma_start(M3[0:1, :], row0[:])
            row1 = small.tile([1, BC], FPR, tag="rowtmp")
            nc.vector.tensor_scalar(out=row1[:], in0=psm_i[:],
                                    scalar1=-1.0, scalar2=None, op0=OP.mult)
            nc.scalar.dma_start(M3[1:2, :], row1[:])
            row2 = small.tile([1, BC], FPR, tag="rowtmp")
            nc.vector.tensor_scalar(out=row2[:], in0=psm_i[:],
                                    scalar1=0.0, scalar2=1.0, op0=OP.mult,
                                    op1=OP.add)
            nc.gpsimd.dma_start(M3[2:3, :], row2[:])
            _stk2.close()

            # ---- Phase C: closed-form 2x2 inverse sqrt, fold gamma -> A
            def stile(tag):
                return small.tile([128, NFT], FP, tag=tag, name=tag)

            arr, bri, cii = stile("arr"), stile("bri"), stile("cii")
            nc.vector.tensor_scalar(out=arr[:], in0=cov[:, 0:NFT],
                                    scalar1=EPS, scalar2=None, op0=OP.add)
            nc.vector.tensor_copy(bri[:], cov[:, NFT:2 * NFT])
            nc.vector.tensor_scalar(out=cii[:], in0=cov[:, 2 * NFT:3 * NFT],
                                    scalar1=EPS, scalar2=None, op0=OP.add)

            det, tmp = stile("det"), stile("tmp")
            nc.vector.tensor_tensor(out=det[:], in0=arr[:], in1=cii[:],
                                    op=OP.mult)
            nc.vector.tensor_tensor(out=tmp[:], in0=bri[:], in1=bri[:],
                                    op=OP.mult)
            nc.vector.tensor_tensor(out=det[:], in0=det[:], in1=tmp[:],
                                    op=OP.subtract)
            s_t = stile("s_t")
            nc.scalar.activation(s_t[:], det[:], AF.Sqrt)
            # tval = sqrt(a + c + 2 s)
            tsum = stile("tsum")
            nc.vector.tensor_tensor(out=tsum[:], in0=arr[:], in1=cii[:],
                                    op=OP.add)
            nc.vector.scalar_tensor_tensor(out=tsum[:], in0=s_t[:], scalar=2.0,
                                           in1=tsum[:], op0=OP.mult, op1=OP.add)
            tval = stile("tval")
            nc.scalar.activation(tval[:], tsum[:], AF.Sqrt)
            den, rden = stile("den"), stile("rden")
            nc.vector.tensor_tensor(out=den[:], in0=s_t[:], in1=tval[:],
                                    op=OP.mult)
            nc.vector.reciprocal(rden[:], den[:])

            w_rr, w_ii, wri_n = stile("w_rr"), stile("w_ii"), stile("wri_n")
            # w_rr = (c+s)*rden ; w_ii = (a+s)*rden ; w_ri = -b*rden = wri_n
            nc.vector.tensor_tensor(out=w_rr[:], in0=cii[:], in1=s_t[:],
                                    op=OP.add)
            nc.vector.tensor_tensor(out=w_rr[:], in0=w_rr[:], in1=rden[:],
                                    op=OP.mult)
            nc.vector.tensor_tensor(out=w_ii[:], in0=arr[:], in1=s_t[:],
                                    op=OP.add)
            nc.vector.tensor_tensor(out=w_ii[:], in0=w_ii[:], in1=rden[:],
                                    op=OP.mult)
            nc.vector.tensor_tensor(out=wri_n[:], in0=bri[:], in1=rden[:],
                                    op=OP.mult)
            nc.vector.tensor_scalar(out=wri_n[:], in0=wri_n[:], scalar1=-1.0,
                                    scalar2=None, op0=OP.mult)

            # A = G @ W,  G = [[g_r, -g_i], [g_i, g_r]], W = [[w_rr, w_ri],
            # [w_ri, w_ii]] with w_ri = wri_n
            def rtile(tag):
                return small.tile([128, NFT], FPR, tag=tag, name=tag)

            a_rr, a_ri = rtile("a_rr"), rtile("a_ri")
            a_ir, a_ii = rtile("a_ir"), rtile("a_ii")
            u, v = stile("u"), stile("v")
            # a_rr = g_r*w_rr - g_i*w_ri
            nc.vector.tensor_tensor(out=u[:], in0=g_r_t[:], in1=w_rr[:],
                                    op=OP.mult)
            nc.vector.tensor_tensor(out=v[:], in0=g_i_t[:], in1=wri_n[:],
                                    op=OP.mult)
            nc.vector.tensor_tensor(out=a_rr[:], in0=u[:], in1=v[:],
                                    op=OP.subtract)
            # a_ri = g_r*w_ri - g_i*w_ii
            nc.vector.tensor_tensor(out=u[:], in0=g_r_t[:], in1=wri_n[:],
                                    op=OP.mult)
            nc.vector.tensor_tensor(out=v[:], in0=g_i_t[:], in1=w_ii[:],
                                    op=OP.mult)
            nc.vector.tensor_tensor(out=a_ri[:], in0=u[:], in1=v[:],
                                    op=OP.subtract)
            # a_ir = g_i*w_rr + g_r*w_ri
            nc.vector.tensor_tensor(out=u[:], in0=g_i_t[:], in1=w_rr[:],
                                    op=OP.mult)
            nc.vector.tensor_tensor(out=v[:], in0=g_r_t[:], in1=wri_n[:],
                                    op=OP.mult)
            nc.vector.tensor_tensor(out=a_ir[:], in0=u[:], in1=v[:],
                                    op=OP.add)
            # a_ii = g_i*w_ri + g_r*w_ii
            nc.vector.tensor_tensor(out=u[:], in0=g_i_t[:], in1=wri_n[:],
                                    op=OP.mult)
            nc.vector.tensor_tensor(out=v[:], in0=g_r_t[:], in1=w_ii[:],
                                    op=OP.mult)
            nc.vector.tensor_tensor(out=a_ii[:], in0=u[:], in1=v[:],
                                    op=OP.add)

            # ---- A3C rhs for the K=3 correction matmul: (3, 4096)
            # row0[2f+c] = (a_rr, a_ir)[c][f]; row1: (a_ri, a_ii); row2: beta
            A3C = small.tile([3, 2 * F], FPR, tag="A3C")
            rings = (nc.sync, nc.scalar, nc.gpsimd, nc.scalar)
            for row, (ev, od) in enumerate(((a_rr, a_ir), (a_ri, a_ii))):
                for cpar, srctile in ((0, ev), (1, od)):
                    # bounce through DRAM; read back in f-major order with a
                    # strided AP.  dram layout: addr(p, t) = 16*p + t.
                    eng = rings[2 * row + cpar]
                    dbuf = dram.tile([128, NFT], FPR, tag=f"dbuf{row}{cpar}",
                                     name=f"dbuf{row}{cpar}")
                    eng.dma_start(dbuf[:], srctile[:])
                    # src iterates (t, p): steps [[1, 16], [16, 128]]
                    src = dbuf[:].rearrange("p t -> (p t)").rearrange(
                        "(p t) -> t p", p=128, t=NFT
                    )
                    dst = A3C[row:row + 1, cpar::2].rearrange(
                        "z (t p) -> z t p", t=NFT, p=128
                    )
                    eng.dma_start(dst, src)
            nc.sync.dma_start(A3C[2:3, :], beta_ilv[:])

            # ---- Phase D: apply.  t-outer; W built on the fly.  Inside each
            # PSUM group the x-matmuls come first (start=True per region) and
            # the A3C correction last, so PE needn't wait for A3C.
            _stk3 = ExitStack()
            ps_o = _stk3.enter_context(
                tc.tile_pool(name="ps_o", bufs=8, space="PSUM"))
            for t2 in range(NFT // 2):
                ta, tb = 2 * t2, 2 * t2 + 1
                Ws = []
                for t in (ta, tb):
                    W_r = wpool.tile([128, 256], BF, tag="W_r",
                                     name=f"W_r_{t}")
                    W_i = wpool.tile([128, 256], BF, tag="W_i",
                                     name=f"W_i_{t}")
                    for W, (ev, od) in ((W_r, (a_rr, a_ir)),
                                        (W_i, (a_ri, a_ii))):
                        Wv = W[:].rearrange("p (g c) -> p g c", c=2)
                        nc.vector.tensor_scalar(
                            out=Wv[:, :, 0], in0=ident_t[:],
                            scalar1=ev[:, t:t + 1].bitcast(FP), scalar2=None,
                            op0=OP.mult,
                        )
                        nc.scalar.activation(
                            Wv[:, :, 1], ident_t[:], AF.Copy,
                            scale=od[:, t:t + 1].bitcast(FP),
                        )
                    Ws.append((W_r, W_i))
                for bh in range(2):
                    stg = stage.tile([128, 4 * 512], FP, tag="stg")
                    pos = []
                    for bb in range(4):
                        b = 4 * bh + bb
                        po = ps_o.tile([128, 512], FP, tag="po")
                        pos.append(po)
                        nc.tensor.matmul(
                            po[:],
                            M3[:, 128 * b:128 * (b + 1)],
                            A3C[:, 512 * t2:512 * (t2 + 1)],
                            start=True, stop=False,
                        )
                        for j, t in enumerate((ta, tb)):
                            W_r, W_i = Ws[j]
                            sl = slice(BC * t + 128 * b,
                                       BC * t + 128 * (b + 1))
                            nc.tensor.matmul(
                                po[:, 256 * j:256 * (j + 1)],
                                xT_r[:, sl], W_r[:],
                                start=False, stop=False,
                            )
                            nc.tensor.matmul(
                                po[:, 256 * j:256 * (j + 1)],
                                xT_i[:, sl], W_i[:],
                                start=False, stop=(j == 1),
                            )
                    for bb in range(4):
                        po = pos[bb]
                        if (4 * bh + bb) % 2 == 0:
                            nc.vector.tensor_copy(
                                stg[:, 512 * bb:512 * (bb + 1)], po[:])
                        else:
                            nc.scalar.copy(
                                stg[:, 512 * bb:512 * (bb + 1)], po[:])
                    # 1 MB store: rows (b, p) -> out[128*b + p, 512*t2:+512]
                    dst = out.rearrange("(a p) f -> p a f", p=128)[
                        :, 4 * bh:4 * (bh + 1), 512 * t2:512 * (t2 + 1)
                    ]
                    src = stg[:].rearrange("p (a q) -> p a q", q=512)
                    seng = (nc.sync, nc.scalar, nc.sync, nc.gpsimd)[
                        (2 * t2 + bh) % 4]
                    seng.dma_start(dst, src)
            _stk3.close()

    split_multi_waits(nc)
    return nc


_CACHE = {}


def _get_nc():
    if "nc" not in _CACHE:
        _CACHE["nc"] = build_bass()
    return _CACHE["nc"]


def _constants():
    if "consts" not in _CACHE:
        sel = np.zeros((128, CSH), dtype=np.float32)
        for p in range(128):
            sel[p, p % CSH] = 1.0
        _CACHE["consts"] = {
            "ident": np.eye(128, dtype=np.float32),
            "identsel": np.ascontiguousarray(np.concatenate(
                [np.eye(128, dtype=np.float32), sel,
                 np.zeros((128, 112), dtype=np.float32)], axis=1)),
            "onesF": np.full((128, 1), 1.0 / F, dtype=np.float32),
        }
    return _CACHE["consts"]


def kernel(x_real, x_imag, gamma_r, gamma_i, beta_r, beta_i):
    x_real = np.ascontiguousarray(x_real, dtype=np.float32)
    x_imag = np.ascontiguousarray(x_imag, dtype=np.float32)
    gamma_r = np.asarray(gamma_r, dtype=np.float32)
    gamma_i = np.asarray(gamma_i, dtype=np.float32)
    beta_r = np.asarray(beta_r, dtype=np.float32)
    beta_i = np.asarray(beta_i, dtype=np.float32)

    nc = _get_nc()
    consts = _constants()
    g_r_t = np.ascontiguousarray(gamma_r.reshape(NFT, 128).T)
    g_i_t = np.ascontiguousarray(gamma_i.reshape(NFT, 128).T)
    beta_ilv = np.ascontiguousarray(
        np.stack([beta_r, beta_i], axis=-1).reshape(1, 2 * F)
    )

    in_maps = []
    for k in range(NCORES):
        cs = slice(CSH * k, CSH * (k + 1))
        in_maps.append({
            "x_r": np.ascontiguousarray(
                x_real[:, cs, :].reshape(BC, F)),
            "x_i": np.ascontiguousarray(
                x_imag[:, cs, :].reshape(BC, F)),
            "g_r": g_r_t, "g_i": g_i_t, "beta_ilv": beta_ilv,
            **consts,
        })

    res = run_bass_kernel_spmd(nc, in_maps, list(range(NCORES)))

    full = np.empty((B, C, F, 2), dtype=np.float32)
    for k in range(NCORES):
        full[:, CSH * k:CSH * (k + 1)] = (
            res.results[k]["out"].reshape(B, CSH, F, 2)
        )
    return full
